# revision 23
# baseline (speedup 1.0000x reference)
"""Cross-attention kernel for Trainium2 (8 NeuronCores, Bass/Tile).

Reference computation (per batch b):
    qproj = query @ W_w.T + W_b          [Q, D]
    scores = qproj @ key.T * (1/sqrt(D)) [Q, K]
    scores = where(mask==0, -inf, scores)
    w = softmax(scores, axis=-1)         [Q, K]
    att = w @ value                      [Q, D]
    returns (att, w)

Sharding: data-parallel over batch (32 batches -> 4 per core x 8 cores).

Device-side layout strategy (everything chains through the PE with no
on-device transposes; matmuls run in bf16, which keeps the PE at the
full 1 column/cycle rate AND enables the compiler's fast-weight-load
path, unlike fp32r whose 4-byte weight loads bound the matmul issue
period; fp32 accumulation in PSUM throughout):
    MM1: qprojT[e,q] = (W^T as lhsT) . (query^T as rhs), bias added in the
         PSUM->SBUF epilogue (per-partition bias broadcast).
    MM2: scoresT[k,q] = (key^T as lhsT) . qprojT, exp fused into the
         epilogue (ScalarE activation with scale=1/sqrt(D), additive mask
         bias per k-partition).
    softmax sum over k: serial running sum across the 8 k-tiles on the DVE
         (each add fires as its exp tile lands), then cross-partition
         reduce+broadcast via GpSimd partition_all_reduce (steady halves,
         latency hidden) or a PE ones-matmul (last half, latency critical);
         reciprocal on VectorE; normalize into a contiguous staging tile
         written out as ONE big SWDGE descriptor per half on the GpSimd
         queue (small per-tile HWDGE writes with 1KB dst lines only
         sustain ~74GB/s and clog the in-order sync queue).
    MM3: att[q,e] = (wT as lhsT) . (value natural layout as rhs), staged
         per q-tile into [128, D] so att descriptors write full 2KB rows.
    A ~3.4us burst of dummy warm-up matmuls at program start flips the PE
    HAM clock gate (1.2 -> 2.4 GHz) before the real stream begins.

The host side only reshapes/transposes/casts (no arithmetic): query/key
are fed pre-transposed per batch in bf16, W is fed transposed, attention
weights come back transposed [K,Q] in bf16 and are untransposed+upcast on
the host.
"""

import numpy as np

B, Q, K, D = 32, 1024, 1024, 1024
N_CORES = 8
BPC = B // N_CORES          # batches per core
SCALE = 1.0 / float(D) ** 0.5
P = 128                     # SBUF partitions
FD = 512                    # matmul moving free dim (psum bank, fp32 out)
QH = Q // FD                # q processed in halves of 512
NEG_BIG = 1.0e30

_CACHE = {}


def _build_nc():
    import concourse.tile as tile
    from concourse import bacc, mybir
    from concourse.bass_isa import ReduceOp

    f32 = mybir.dt.float32
    bf16 = mybir.dt.bfloat16
    AF = mybir.ActivationFunctionType
    ALU = mybir.AluOpType

    nc = bacc.Bacc("TRN2", target_bir_lowering=False, debug=False,
                   num_devices=N_CORES)

    # host pre-tiled layouts: [P, tile*cols] so every DMA is a plain 2D
    # copy with long contiguous per-partition rows (qT partition p, column
    # d*Q+q holds query[b, q, d*128+p], etc.)
    qT = nc.dram_tensor("qT", [BPC, P, (D // P) * Q], bf16,
                        kind="ExternalInput").ap()
    kT = nc.dram_tensor("kT", [BPC, P, (D // P) * K], bf16,
                        kind="ExternalInput").ap()
    v = nc.dram_tensor("v", [BPC, P, (K // P) * D], bf16,
                       kind="ExternalInput").ap()
    wT = nc.dram_tensor("wT", [D, D], bf16, kind="ExternalInput").ap()
    bias = nc.dram_tensor("bias", [D], f32, kind="ExternalInput").ap()
    mask = nc.dram_tensor("mask", [BPC, K], f32, kind="ExternalInput").ap()
    att = nc.dram_tensor("att", [BPC, Q, D], bf16, kind="ExternalOutput").ap()
    aw = nc.dram_tensor("aw", [BPC, K, Q], bf16, kind="ExternalOutput").ap()

    DT = D // P   # d/e/k tiles of 128
    KT = K // P

    with tile.TileContext(nc) as tc:
        with (
            tc.tile_pool(name="consts", bufs=1) as consts,
            tc.tile_pool(name="wt", bufs=DT) as wt_pool,
            tc.tile_pool(name="qt", bufs=DT) as qt_pool,
            tc.tile_pool(name="kt", bufs=2) as kt_pool,
            tc.tile_pool(name="vv", bufs=2) as v_pool,
            tc.tile_pool(name="qp", bufs=DT) as qp_pool,
            tc.tile_pool(name="ex", bufs=KT) as ex_pool,
            tc.tile_pool(name="mb", bufs=2) as mb_pool,
            tc.tile_pool(name="rs", bufs=2) as rs_pool,
            tc.tile_pool(name="st", bufs=6) as st_pool,
            tc.tile_pool(name="aw", bufs=1) as aw_pool,
            tc.tile_pool(name="ao", bufs=8) as att_pool,
            tc.tile_pool(name="psum", bufs=8, space="PSUM") as psum_pool,
        ):
            # constants: bias (per-partition layout), ones for the
            # partition-dim softmax sum.  Batch 0's W^T/q/k tiles are DMA'd
            # interleaved (emit_batch_dmas below) so the d-outer first MM1
            # starts after <1MB of DMA and MM2 isn't starved on kT.
            # ones tile via a direct bf16 GpSimd memset, emitted FIRST so
            # the PE warm-up below is not gated on the Vector engine's
            # late preamble (memset bit-packs 1.0 per-dtype, so a direct
            # bf16 memset is exact)
            ones_sb = consts.tile([P, P], bf16, name="ones")
            nc.gpsimd.memset(ones_sb[:], 1.0)

            # HAM warm-up: the PE clock-gate sits at 4/8 (1.2 GHz) until it
            # has seen ~3.4us of sustained matmul activity.  The real MM1
            # can't start until ~3us of wT/qT DMA has landed, so burn that
            # wait on dummy matmuls (memset tile, never read) to (a) flip
            # the HAM to 8/8 before real work and (b) not leave the PE idle.
            warm_ps = psum_pool.tile([P, FD], f32, name="ps")
            for _ in range(32):
                nc.tensor.matmul(warm_ps[:, 0:P], ones_sb[:], ones_sb[:],
                                 start=True, stop=True)

            wt_sb = []
            bias_sb = consts.tile([P, DT], f32, name="bias")
            nc.gpsimd.dma_start(bias_sb[:], bias.rearrange("(a b) -> b a", b=P))

            NH = BPC * QH          # total half-iterations on this core

            state = {}             # per-batch tiles: qt/kt/v/mbias

            def emit_batch_dmas(b):
                # tiny mask transfer goes on the GpSimd DMA queue so it is
                # not stuck behind this batch's bulk input DMA
                mraw = mb_pool.tile([P, KT], f32, name="mraw")
                nc.gpsimd.dma_start(mraw[:], mask[b].rearrange("(a b) -> b a", b=P))
                mbias = mb_pool.tile([P, KT], f32, name="mbias")
                nc.vector.tensor_scalar(mbias[:], mraw[:], NEG_BIG, -NEG_BIG,
                                        ALU.mult, ALU.add)
                # kT and v move as ONE descriptor each (descriptor issue on
                # the queue engine costs ~0.6us apiece, so 8 separate tile
                # DMAs would delay MM2's inputs by ~5us at startup); wT/qT
                # stay per-tile so the d-outer MM1 starts on the first pair.
                qt_sb = []
                kt_all = kt_pool.tile([P, DT * K], bf16, name="kt")
                if b == 0:
                    # startup-critical order: (wt[d], qt[d]-first-half) pairs
                    # pace the d-outer MM1 for half 0 (descriptor issue on the
                    # queue engine is ~0.6us apiece, so halving the qt payload
                    # per pair pulls the first matmul ~2us earlier); kt tiles
                    # follow for the e-outer MM2, then the qt second halves
                    # (first needed by MM1 of half 1, ~50us in), then v.
                    for d in range(DT):
                        t = wt_pool.tile([P, D], bf16, name="wt")
                        nc.sync.dma_start(t[:], wT[d * P:(d + 1) * P, :])
                        wt_sb.append(t)
                        t = qt_pool.tile([P, Q], bf16, name="qt")
                        nc.sync.dma_start(t[:, 0:FD], qT[b, :, d * Q:d * Q + FD])
                        qt_sb.append(t)
                    for e in range(DT):
                        nc.sync.dma_start(kt_all[:, e * K:(e + 1) * K],
                                          kT[b, :, e * K:(e + 1) * K])
                    for d in range(DT):
                        nc.sync.dma_start(qt_sb[d][:, FD:Q],
                                          qT[b, :, d * Q + FD:(d + 1) * Q])
                else:
                    nc.sync.dma_start(kt_all[:], kT[b])
                    for d in range(DT):
                        t = qt_pool.tile([P, Q], bf16, name="qt")
                        nc.sync.dma_start(t[:], qT[b, :, d * Q:(d + 1) * Q])
                        qt_sb.append(t)
                v_all = v_pool.tile([P, KT * D], bf16, name="vv")
                nc.sync.dma_start(v_all[:], v[b])
                state[b] = (qt_sb, kt_all, v_all, mbias)

            def emit_mm1_group(hi, et):
                """One MM1 accumulation group: qprojT[e-tile et, half hi]."""
                b, qh = divmod(hi, QH)
                qt_sb = state[b][0]
                qs = slice(qh * FD, (qh + 1) * FD)
                ps = psum_pool.tile([P, FD], f32, name="ps")
                for d in range(DT):
                    nc.tensor.matmul(
                        ps[:],
                        wt_sb[d][:, et * P:(et + 1) * P],
                        qt_sb[d][:, qs],
                        start=(d == 0), stop=(d == DT - 1),
                    )
                o = qp_pool.tile([P, FD], bf16, name="qp")
                nc.scalar.activation(o[:], ps[:], AF.Identity,
                                     bias=bias_sb[:, et:et + 1], scale=1.0)
                return o

            def emit_mm1_douter(hi):
                """MM1 with the d-contraction as the outer loop: the first
                matmuls only need the first wT/qT d-tiles, so the PE starts
                as soon as ~0.75MB of DMA has landed (used for hi==0)."""
                b, qh = divmod(hi, QH)
                qt_sb = state[b][0]
                qs = slice(qh * FD, (qh + 1) * FD)
                pss = [psum_pool.tile([P, FD], f32, name="ps")
                       for _ in range(DT)]
                for d in range(DT):
                    for et in range(DT):
                        nc.tensor.matmul(
                            pss[et][:],
                            wt_sb[d][:, et * P:(et + 1) * P],
                            qt_sb[d][:, qs],
                            start=(d == 0), stop=(d == DT - 1),
                        )
                qp_sb = []
                for et in range(DT):
                    o = qp_pool.tile([P, FD], bf16, name="qp")
                    nc.scalar.activation(o[:], pss[et][:], AF.Identity,
                                         bias=bias_sb[:, et:et + 1], scale=1.0)
                    qp_sb.append(o)
                return qp_sb

            # ---------------- software-pipelined half-iterations ----------
            # PE stream per half hi:
            #   MM2(hi), [MM1(hi+1) et0], sum(hi), [MM1(hi+1) et1..7], MM3(hi)
            # so the reciprocal+normalize of half hi runs on the DVE while
            # the PE chews through MM1 of half hi+1 -> no PE gaps.
            emit_batch_dmas(0)
            qp_cur = emit_mm1_douter(0)

            for hi in range(NH):
                b, qh = divmod(hi, QH)
                qs = slice(qh * FD, (qh + 1) * FD)
                _, kt_all, v_all, mbias = state[b]

                # ---- MM2: scoresT[k, q-half] -> exp ----
                ex_sb = []
                if hi == 0:
                    # e-outer accumulation across all 8 PSUM banks: each
                    # e-pass needs only kt tile e, so MM2 starts while the
                    # tail of the kT DMA is still in flight.
                    pss = [psum_pool.tile([P, FD], f32, name="ps")
                           for _ in range(KT)]
                    for e in range(DT):
                        for kt_i in range(KT):
                            nc.tensor.matmul(
                                pss[kt_i][:],
                                kt_all[:, e * K + kt_i * P:
                                       e * K + (kt_i + 1) * P],
                                qp_cur[e][:],
                                start=(e == 0), stop=(e == DT - 1),
                            )
                    for kt_i in range(KT):
                        o = ex_pool.tile([P, FD], bf16, name="ex")
                        nc.scalar.activation(o[:], pss[kt_i][:], AF.Exp,
                                             bias=mbias[:, kt_i:kt_i + 1],
                                             scale=SCALE)
                        ex_sb.append(o)
                else:
                    for kt_i in range(KT):
                        ps = psum_pool.tile([P, FD], f32, name="ps")
                        for e in range(DT):
                            nc.tensor.matmul(
                                ps[:],
                                kt_all[:, e * K + kt_i * P:
                                       e * K + (kt_i + 1) * P],
                                qp_cur[e][:],
                                start=(e == 0), stop=(e == DT - 1),
                            )
                        o = ex_pool.tile([P, FD], bf16, name="ex")
                        nc.scalar.activation(o[:], ps[:], AF.Exp,
                                             bias=mbias[:, kt_i:kt_i + 1],
                                             scale=SCALE)
                        ex_sb.append(o)

                nxt = hi + 1
                if nxt < NH and nxt % QH == 0:
                    emit_batch_dmas(nxt // QH)

                # ---- softmax sum over the 8 k-tiles: SERIAL running sum on
                # the DVE (each add fires as soon as its exp tile lands, so
                # the final sum trails the LAST exp by one add instead of a
                # tree's two pending levels), then a single ones-matmul to
                # reduce+broadcast across partitions.
                run = rs_pool.tile([P, FD], f32, name="sm0")
                nc.vector.tensor_add(run[:], ex_sb[0][:], ex_sb[1][:])
                for i in range(2, KT - 1):
                    nxt_run = rs_pool.tile([P, FD], f32, name=f"sm{i}")
                    nc.vector.tensor_add(nxt_run[:], run[:], ex_sb[i][:])
                    run = nxt_run
                tsum = rs_pool.tile([P, FD], bf16, name="smf")
                nc.vector.tensor_add(tsum[:], run[:], ex_sb[KT - 1][:])

                if nxt < NH:
                    # steady state: MM1 of the next half covers the PE while
                    # the softmax reduce + reciprocal + normalize run on
                    # GpSimd/DVE (the cross-partition sum goes to GpSimd's
                    # partition_all_reduce instead of a PE ones-matmul,
                    # shaving 512 PE cycles per half; its latency is hidden
                    # behind MM1-next + MM3)
                    qp_next = []
                    for et in range(3):
                        qp_next.append(emit_mm1_group(nxt, et))

                    rsum = rs_pool.tile([P, FD], f32, name="rsum")
                    nc.gpsimd.partition_all_reduce(rsum[:], tsum[:], P,
                                                   ReduceOp.add)
                    rs = rs_pool.tile([P, FD], f32, name="rs")
                    nc.vector.reciprocal_approx_fast(rs[:], rsum[:])

                    for et in range(3, DT):
                        qp_next.append(emit_mm1_group(nxt, et))

                    # normalize into ONE contiguous staging tile (hidden
                    # behind MM1-next on the PE), then a single big SWDGE
                    # descriptor on the otherwise-idle GpSimd queue.  Small
                    # per-tile writes (1KB dst lines) only sustain ~74GB/s
                    # and 8 of them per half saturate the in-order sync
                    # queue, starving the next batch's input DMAs.
                    aw_st = aw_pool.tile([P, KT * FD], bf16, name="awst",
                                         bufs=2)
                    for kt_i in range(KT):
                        nc.vector.tensor_mul(
                            aw_st[:, kt_i * FD:(kt_i + 1) * FD],
                            ex_sb[kt_i][:], rs[:])
                    nc.gpsimd.dma_start(
                        aw[b].rearrange("(kt p) q -> p kt q", p=P)[:, :, qs],
                        aw_st[:].rearrange("p (kt q) -> p kt q", q=FD))

                    # MM3: att[q-half, e] on normalized weights, staged per
                    # q-tile into [P, D] so the att descriptor writes full
                    # 2KB dram rows (4 descriptors per half instead of 8)
                    for qt_i in range(FD // P):
                        ao = att_pool.tile([P, D], bf16, name="ao")
                        for ec in range(D // FD):
                            ps = psum_pool.tile([P, FD], f32, name="ps")
                            for kt_i in range(KT):
                                nc.tensor.matmul(
                                    ps[:],
                                    aw_st[:, kt_i * FD + qt_i * P:
                                          kt_i * FD + (qt_i + 1) * P],
                                    v_all[:, kt_i * D + ec * FD:
                                          kt_i * D + (ec + 1) * FD],
                                    start=(kt_i == 0), stop=(kt_i == KT - 1),
                                )
                            nc.vector.tensor_copy(
                                ao[:, ec * FD:(ec + 1) * FD], ps[:])
                        q0 = qh * FD + qt_i * P
                        nc.sync.dma_start(att[b, q0:q0 + P, :], ao[:])

                    qp_cur = qp_next
                else:
                    # last half: there is no next MM1 to hide the softmax
                    # critical path behind, so run MM3 on the UNNORMALIZED
                    # weights immediately (only depends on exp), and fold
                    # the 1/sum scale into the SBUF epilogue as a
                    # per-q-partition multiply.  The drain of this half IS
                    # the kernel tail, so everything is staged into few big
                    # output descriptors (descriptor issue costs ~0.6us
                    # apiece on the queue engine) and the aw normalize is
                    # split across DVE+GpSimd with its DMA on the (idle)
                    # GpSimd SWDGE queue.
                    def emit_mm3u_ps(qt_i, ec):
                        ps = psum_pool.tile([P, FD], f32, name="ps")
                        for kt_i in range(KT):
                            nc.tensor.matmul(
                                ps[:],
                                ex_sb[kt_i][:, qt_i * P:(qt_i + 1) * P],
                                v_all[:, kt_i * D + ec * FD:
                                      kt_i * D + (ec + 1) * FD],
                                start=(kt_i == 0), stop=(kt_i == KT - 1),
                            )
                        return ps

                    groups = [(qt_i, ec) for qt_i in range(FD // P)
                              for ec in range(D // FD)]
                    sts = []
                    # one MM3 group first: by its end the DVE add-tree has
                    # produced tsum, so the sum-matmul below runs gap-free.
                    ps0 = emit_mm3u_ps(*groups[0])
                    st0 = st_pool.tile([P, FD], f32, name="st")
                    nc.vector.tensor_copy(st0[:], ps0[:])
                    sts.append((st0,) + groups[0])

                    # softmax scale factors, emitted early so the PE computes
                    # them between MM3 groups and the DVE-side scaling +
                    # output DMA overlaps the remaining MM3 groups
                    ps = psum_pool.tile([P, FD], f32, name="ps")
                    nc.tensor.matmul(ps[:], ones_sb[:], tsum[:],
                                     start=True, stop=True)
                    rs = rs_pool.tile([P, FD], f32, name="rs")
                    nc.vector.reciprocal_approx_fast(rs[:], ps[:])

                    ps1 = emit_mm3u_ps(*groups[1])
                    st1 = st_pool.tile([P, FD], f32, name="st")
                    nc.vector.tensor_copy(st1[:], ps1[:])
                    sts.append((st1,) + groups[1])

                    # weights normalize into ONE contiguous staging tile,
                    # all on the DVE (GpSimd tensor ops are ~4x slower and
                    # would gate the aw descriptor), emitted BEFORE the rc
                    # copies/att scale-outs so the big SWDGE aw descriptor
                    # generates as early as possible -- the end-of-kernel
                    # barrier waits for this transfer.  NOT in place: later
                    # MM3 groups still read the unnormalized ex tiles.
                    aw_st = aw_pool.tile([P, KT * FD], bf16, name="awst",
                                         bufs=2)
                    for kt_i in range(KT):
                        nc.vector.tensor_mul(
                            aw_st[:, kt_i * FD:(kt_i + 1) * FD],
                            ex_sb[kt_i][:], rs[:])
                    nc.gpsimd.dma_start(
                        aw[b].rearrange("(kt p) q -> p kt q", p=P)[:, :, qs],
                        aw_st[:].rearrange("p (kt q) -> p kt q", q=FD))

                    # per-q-partition 1/sum COLUMNS for the att scale: a
                    # tiny N=1 matmul per q-tile, (tsum_slice)^T @ ones_col
                    # = sum(q) in column layout (~60 cycles each, vs ~660
                    # for a PE transpose of the row-form reciprocal), then
                    # a [P,1] reciprocal on the DVE
                    rc_sb = []
                    for qt_i in range(FD // P):
                        pst = psum_pool.tile([P, FD], f32, name="ps")
                        nc.tensor.matmul(pst[:, 0:1],
                                         tsum[:, qt_i * P:(qt_i + 1) * P],
                                         ones_sb[:, 0:1],
                                         start=True, stop=True)
                        rc = rs_pool.tile([P, 1], f32, name="rc", bufs=4)
                        nc.vector.reciprocal_approx_fast(rc[:], pst[:, 0:1])
                        rc_sb.append(rc)

                    # att staged per q-tile ([128, D] covering both ec
                    # halves) -> 4 descriptors of 256KB instead of 8x128KB
                    att_st = [att_pool.tile([P, D], bf16, name="ao")
                              for i in range(FD // P)]

                    def emit_scale_out(src, qt_i, ec):
                        nc.vector.tensor_scalar_mul(
                            att_st[qt_i][:, ec * FD:(ec + 1) * FD], src[:],
                            rc_sb[qt_i][:, 0:1])
                        if ec == D // FD - 1:
                            # qt2 rides the (already-drained) SWDGE queue so
                            # the final tile qt3 doesn't serialize behind it
                            # on the sync queue; qt3 stays on sync because
                            # SWDGE descriptor-gen costs ~2us and would gate
                            # the very last bytes
                            eng = nc.gpsimd if qt_i == 2 else nc.sync
                            q0 = qh * FD + qt_i * P
                            eng.dma_start(att[b, q0:q0 + P, :],
                                          att_st[qt_i][:])

                    for st, qt_i, ec in sts:
                        emit_scale_out(st, qt_i, ec)
                    for qt_i, ec in groups[2:]:
                        # rc is ready by now: scale straight out of PSUM in
                        # a single DVE pass (no staging copy)
                        ps = emit_mm3u_ps(qt_i, ec)
                        emit_scale_out(ps, qt_i, ec)
    nc.compile()
    return nc


def _get_nc():
    if "nc" not in _CACHE:
        _CACHE["nc"] = _build_nc()
    return _CACHE["nc"]


def _make_in_maps(query, key, value, mask, W_w, W_b):
    import ml_dtypes
    bf16 = ml_dtypes.bfloat16

    query = np.asarray(query, dtype=np.float32)
    key = np.asarray(key, dtype=np.float32)
    value = np.asarray(value, dtype=np.float32)
    W_w = np.asarray(W_w, dtype=np.float32)
    W_b = np.ascontiguousarray(W_b, dtype=np.float32)
    mask_f = np.ascontiguousarray(mask, dtype=np.float32)

    # host-side layout prep only (transposes / casts, no arithmetic):
    # [B, P, tiles*cols] pre-tiled layouts, see _build_nc
    qT = np.ascontiguousarray(
        query.transpose(0, 2, 1).reshape(B, D // P, P, Q)
        .transpose(0, 2, 1, 3).reshape(B, P, (D // P) * Q)).astype(bf16)
    kT = np.ascontiguousarray(
        key.transpose(0, 2, 1).reshape(B, D // P, P, K)
        .transpose(0, 2, 1, 3).reshape(B, P, (D // P) * K)).astype(bf16)
    v_b = np.ascontiguousarray(
        value.reshape(B, K // P, P, D)
        .transpose(0, 2, 1, 3).reshape(B, P, (K // P) * D)).astype(bf16)
    wT = np.ascontiguousarray(W_w.T).astype(bf16)                  # [Din,Dout]

    in_maps = []
    for c in range(N_CORES):
        sl = slice(c * BPC, (c + 1) * BPC)
        in_maps.append({
            "qT": qT[sl], "kT": kT[sl], "v": v_b[sl],
            "wT": wT, "bias": W_b, "mask": mask_f[sl],
        })
    return in_maps


def kernel(query, key, value, mask, W_w, W_b):
    from concourse.bass_utils import run_bass_kernel_spmd

    nc = _get_nc()
    in_maps = _make_in_maps(query, key, value, mask, W_w, W_b)

    def _axon_reset():
        try:
            import ctypes
            lib = ctypes.CDLL("/opt/axon/libaxon_pjrt.so")
            if hasattr(lib, "axon_reset"):
                lib.axon_reset.restype = ctypes.c_int64
                lib.axon_reset()
        except Exception:
            pass

    att = weights = None
    for _attempt in range(3):
        try:
            res = run_bass_kernel_spmd(nc, in_maps,
                                       core_ids=list(range(N_CORES)))
        except Exception:
            if _attempt == 2:
                raise
            _axon_reset()
            continue
        att = np.concatenate(
            [res.results[c]["att"].astype(np.float32) for c in range(N_CORES)],
            axis=0)
        awT = np.concatenate(
            [res.results[c]["aw"].astype(np.float32) for c in range(N_CORES)],
            axis=0)
        weights = np.ascontiguousarray(awT.transpose(0, 2, 1))  # [B, Q, K]
        # sanity check (guards against rare cold-start misexecution):
        # sampled softmax rows must sum to ~1 and outputs must be finite
        row_sums = weights[:, ::97, :].sum(axis=-1)
        if (np.all(np.abs(row_sums - 1.0) < 5e-2)
                and np.isfinite(att).all()):
            break
    return att, weights



# revision 26
# speedup vs baseline: 1.0042x; 1.0042x over previous
"""Cross-attention kernel for Trainium2 (8 NeuronCores, Bass/Tile).

Reference computation (per batch b):
    qproj = query @ W_w.T + W_b          [Q, D]
    scores = qproj @ key.T * (1/sqrt(D)) [Q, K]
    scores = where(mask==0, -inf, scores)
    w = softmax(scores, axis=-1)         [Q, K]
    att = w @ value                      [Q, D]
    returns (att, w)

Sharding: data-parallel over batch (32 batches -> 4 per core x 8 cores).

Device-side layout strategy (everything chains through the PE with no
on-device transposes; matmuls run in bf16, which keeps the PE at the
full 1 column/cycle rate AND enables the compiler's fast-weight-load
path, unlike fp32r whose 4-byte weight loads bound the matmul issue
period; fp32 accumulation in PSUM throughout):
    MM1: qprojT[e,q] = (W^T as lhsT) . (query^T as rhs), bias added in the
         PSUM->SBUF epilogue (per-partition bias broadcast).
    MM2: scoresT[k,q] = (key^T as lhsT) . qprojT, exp fused into the
         epilogue (ScalarE activation with scale=1/sqrt(D), additive mask
         bias per k-partition).
    softmax sum over k: serial running sum across the 8 k-tiles on the DVE
         (each add fires as its exp tile lands), then cross-partition
         reduce+broadcast via GpSimd partition_all_reduce (steady halves,
         latency hidden) or a PE ones-matmul (last half, latency critical);
         reciprocal on VectorE; normalize into a contiguous staging tile
         written out as ONE big SWDGE descriptor per half on the GpSimd
         queue (small per-tile HWDGE writes with 1KB dst lines only
         sustain ~74GB/s and clog the in-order sync queue).
    MM3: att[q,e] = (wT as lhsT) . (value natural layout as rhs), staged
         per q-tile into [128, D] so att descriptors write full 2KB rows.
    A ~3.4us burst of dummy warm-up matmuls at program start flips the PE
    HAM clock gate (1.2 -> 2.4 GHz) before the real stream begins.

The host side only reshapes/transposes/casts (no arithmetic): query/key
are fed pre-transposed per batch in bf16, W is fed transposed, attention
weights come back transposed [K,Q] in bf16 and are untransposed+upcast on
the host.
"""

import numpy as np

B, Q, K, D = 32, 1024, 1024, 1024
N_CORES = 8
BPC = B // N_CORES          # batches per core
SCALE = 1.0 / float(D) ** 0.5
P = 128                     # SBUF partitions
FD = 512                    # matmul moving free dim (psum bank, fp32 out)
QH = Q // FD                # q processed in halves of 512
NEG_BIG = 1.0e30

_CACHE = {}


def _build_nc():
    import concourse.tile as tile
    from concourse import bacc, mybir
    from concourse.bass_isa import ReduceOp

    f32 = mybir.dt.float32
    bf16 = mybir.dt.bfloat16
    AF = mybir.ActivationFunctionType
    ALU = mybir.AluOpType

    nc = bacc.Bacc("TRN2", target_bir_lowering=False, debug=False,
                   num_devices=N_CORES)

    # host pre-tiled layouts: [P, tile*cols] so every DMA is a plain 2D
    # copy with long contiguous per-partition rows (qT partition p, column
    # d*Q+q holds query[b, q, d*128+p], etc.)
    qT = nc.dram_tensor("qT", [BPC, P, (D // P) * Q], bf16,
                        kind="ExternalInput").ap()
    kT = nc.dram_tensor("kT", [BPC, P, (D // P) * K], bf16,
                        kind="ExternalInput").ap()
    v = nc.dram_tensor("v", [BPC, P, (K // P) * D], bf16,
                       kind="ExternalInput").ap()
    wT = nc.dram_tensor("wT", [D, D], bf16, kind="ExternalInput").ap()
    bias = nc.dram_tensor("bias", [D], f32, kind="ExternalInput").ap()
    mask = nc.dram_tensor("mask", [BPC, K], f32, kind="ExternalInput").ap()
    att = nc.dram_tensor("att", [BPC, Q, D], bf16, kind="ExternalOutput").ap()
    aw = nc.dram_tensor("aw", [BPC, K, Q], bf16, kind="ExternalOutput").ap()

    DT = D // P   # d/e/k tiles of 128
    KT = K // P

    with tile.TileContext(nc) as tc:
        with (
            tc.tile_pool(name="consts", bufs=1) as consts,
            tc.tile_pool(name="wt", bufs=DT) as wt_pool,
            tc.tile_pool(name="qt", bufs=DT) as qt_pool,
            tc.tile_pool(name="kt", bufs=2) as kt_pool,
            tc.tile_pool(name="vv", bufs=2) as v_pool,
            tc.tile_pool(name="qp", bufs=DT) as qp_pool,
            tc.tile_pool(name="ex", bufs=KT) as ex_pool,
            tc.tile_pool(name="mb", bufs=2) as mb_pool,
            tc.tile_pool(name="rs", bufs=2) as rs_pool,
            tc.tile_pool(name="st", bufs=6) as st_pool,
            tc.tile_pool(name="aw", bufs=1) as aw_pool,
            tc.tile_pool(name="ao", bufs=8) as att_pool,
            tc.tile_pool(name="psum", bufs=8, space="PSUM") as psum_pool,
        ):
            # constants: bias (per-partition layout), ones for the
            # partition-dim softmax sum.  Batch 0's W^T/q/k tiles are DMA'd
            # interleaved (emit_batch_dmas below) so the d-outer first MM1
            # starts after <1MB of DMA and MM2 isn't starved on kT.
            # ones tile via a direct bf16 GpSimd memset, emitted FIRST so
            # the PE warm-up below is not gated on the Vector engine's
            # late preamble (memset bit-packs 1.0 per-dtype, so a direct
            # bf16 memset is exact)
            ones_sb = consts.tile([P, P], bf16, name="ones")
            nc.gpsimd.memset(ones_sb[:], 1.0)

            # HAM warm-up: the PE clock-gate sits at 4/8 (1.2 GHz) until it
            # has seen ~3.4us of sustained matmul activity.  The real MM1
            # can't start until ~3us of wT/qT DMA has landed, so burn that
            # wait on dummy matmuls (memset tile, never read) to (a) flip
            # the HAM to 8/8 before real work and (b) not leave the PE idle.
            warm_ps = psum_pool.tile([P, FD], f32, name="ps")
            for _ in range(32):
                nc.tensor.matmul(warm_ps[:, 0:P], ones_sb[:], ones_sb[:],
                                 start=True, stop=True)

            wt_sb = []
            bias_sb = consts.tile([P, DT], f32, name="bias")
            nc.gpsimd.dma_start(bias_sb[:], bias.rearrange("(a b) -> b a", b=P))

            NH = BPC * QH          # total half-iterations on this core

            state = {}             # per-batch tiles: qt/kt/v/mbias

            def emit_batch_dmas(b):
                # tiny mask transfer goes on the GpSimd DMA queue so it is
                # not stuck behind this batch's bulk input DMA
                mraw = mb_pool.tile([P, KT], f32, name="mraw")
                nc.gpsimd.dma_start(mraw[:], mask[b].rearrange("(a b) -> b a", b=P))
                mbias = mb_pool.tile([P, KT], f32, name="mbias")
                nc.vector.tensor_scalar(mbias[:], mraw[:], NEG_BIG, -NEG_BIG,
                                        ALU.mult, ALU.add)
                # kT and v move as ONE descriptor each (descriptor issue on
                # the queue engine costs ~0.6us apiece, so 8 separate tile
                # DMAs would delay MM2's inputs by ~5us at startup); wT/qT
                # stay per-tile so the d-outer MM1 starts on the first pair.
                qt_sb = []
                kt_all = kt_pool.tile([P, DT * K], bf16, name="kt")
                if b == 0:
                    # startup-critical order: (wt[d], qt[d]-first-half) pairs
                    # pace the d-outer MM1 for half 0 (descriptor issue on the
                    # queue engine is ~0.6us apiece, so halving the qt payload
                    # per pair pulls the first matmul ~2us earlier); kt tiles
                    # follow for the e-outer MM2, then the qt second halves
                    # (first needed by MM1 of half 1, ~50us in), then v.
                    # d=0 is quartered further: the DMA ring's first ~4us of
                    # packets run ~5x slow, and the split d=0 sub-passes below
                    # only need 128KB+64KB to start the PE.
                    for d in range(DT):
                        t = wt_pool.tile([P, D], bf16, name="wt")
                        if d == 0:
                            nc.sync.dma_start(t[:, 0:FD],
                                              wT[0:P, 0:FD])
                            tq = qt_pool.tile([P, Q], bf16, name="qt")
                            nc.sync.dma_start(tq[:, 0:FD // 2],
                                              qT[b, :, 0:FD // 2])
                            nc.sync.dma_start(t[:, FD:D],
                                              wT[0:P, FD:D])
                            nc.sync.dma_start(tq[:, FD // 2:FD],
                                              qT[b, :, FD // 2:FD])
                            wt_sb.append(t)
                            qt_sb.append(tq)
                            continue
                        nc.sync.dma_start(t[:], wT[d * P:(d + 1) * P, :])
                        wt_sb.append(t)
                        t = qt_pool.tile([P, Q], bf16, name="qt")
                        nc.sync.dma_start(t[:, 0:FD], qT[b, :, d * Q:d * Q + FD])
                        qt_sb.append(t)
                    for e in range(DT):
                        nc.sync.dma_start(kt_all[:, e * K:(e + 1) * K],
                                          kT[b, :, e * K:(e + 1) * K])
                    for d in range(DT):
                        nc.sync.dma_start(qt_sb[d][:, FD:Q],
                                          qT[b, :, d * Q + FD:(d + 1) * Q])
                else:
                    nc.sync.dma_start(kt_all[:], kT[b])
                    for d in range(DT):
                        t = qt_pool.tile([P, Q], bf16, name="qt")
                        nc.sync.dma_start(t[:], qT[b, :, d * Q:(d + 1) * Q])
                        qt_sb.append(t)
                v_all = v_pool.tile([P, KT * D], bf16, name="vv")
                nc.sync.dma_start(v_all[:], v[b])
                state[b] = (qt_sb, kt_all, v_all, mbias)

            def emit_mm1_group(hi, et):
                """One MM1 accumulation group: qprojT[e-tile et, half hi]."""
                b, qh = divmod(hi, QH)
                qt_sb = state[b][0]
                qs = slice(qh * FD, (qh + 1) * FD)
                ps = psum_pool.tile([P, FD], f32, name="ps")
                for d in range(DT):
                    nc.tensor.matmul(
                        ps[:],
                        wt_sb[d][:, et * P:(et + 1) * P],
                        qt_sb[d][:, qs],
                        start=(d == 0), stop=(d == DT - 1),
                    )
                o = qp_pool.tile([P, FD], bf16, name="qp")
                nc.scalar.activation(o[:], ps[:], AF.Identity,
                                     bias=bias_sb[:, et:et + 1], scale=1.0)
                return o

            def emit_mm1_douter(hi):
                """MM1 with the d-contraction as the outer loop: the first
                matmuls only need the first wT/qT d-tiles, so the PE starts
                as soon as ~0.2MB of DMA has landed (used for hi==0).  The
                d=0 pass runs as two half-width column sub-passes gated on
                the quartered qt0 descriptors; PSUM's per-element
                has_written bit makes start=True on sub-pass 0 + start=False
                on sub-pass 1 equivalent to one full-width start=True."""
                b, qh = divmod(hi, QH)
                qt_sb = state[b][0]
                qs = slice(qh * FD, (qh + 1) * FD)
                pss = [psum_pool.tile([P, FD], f32, name="ps")
                       for _ in range(DT)]
                HF = FD // 2
                for half in range(2):
                    cs = slice(half * HF, (half + 1) * HF)
                    qcs = slice(qh * FD + half * HF,
                                qh * FD + (half + 1) * HF)
                    for et in range(DT):
                        nc.tensor.matmul(
                            pss[et][:, cs],
                            wt_sb[0][:, et * P:(et + 1) * P],
                            qt_sb[0][:, qcs],
                            start=(half == 0), stop=False,
                        )
                for d in range(1, DT):
                    for et in range(DT):
                        nc.tensor.matmul(
                            pss[et][:],
                            wt_sb[d][:, et * P:(et + 1) * P],
                            qt_sb[d][:, qs],
                            start=False, stop=(d == DT - 1),
                        )
                qp_sb = []
                for et in range(DT):
                    o = qp_pool.tile([P, FD], bf16, name="qp")
                    nc.scalar.activation(o[:], pss[et][:], AF.Identity,
                                         bias=bias_sb[:, et:et + 1], scale=1.0)
                    qp_sb.append(o)
                return qp_sb

            # ---------------- software-pipelined half-iterations ----------
            # PE stream per half hi:
            #   MM2(hi), [MM1(hi+1) et0], sum(hi), [MM1(hi+1) et1..7], MM3(hi)
            # so the reciprocal+normalize of half hi runs on the DVE while
            # the PE chews through MM1 of half hi+1 -> no PE gaps.
            emit_batch_dmas(0)
            qp_cur = emit_mm1_douter(0)

            for hi in range(NH):
                b, qh = divmod(hi, QH)
                qs = slice(qh * FD, (qh + 1) * FD)
                _, kt_all, v_all, mbias = state[b]

                # ---- MM2: scoresT[k, q-half] -> exp ----
                ex_sb = []
                if hi == 0:
                    # e-outer accumulation across all 8 PSUM banks: each
                    # e-pass needs only kt tile e, so MM2 starts while the
                    # tail of the kT DMA is still in flight.
                    pss = [psum_pool.tile([P, FD], f32, name="ps")
                           for _ in range(KT)]
                    for e in range(DT):
                        for kt_i in range(KT):
                            nc.tensor.matmul(
                                pss[kt_i][:],
                                kt_all[:, e * K + kt_i * P:
                                       e * K + (kt_i + 1) * P],
                                qp_cur[e][:],
                                start=(e == 0), stop=(e == DT - 1),
                            )
                    for kt_i in range(KT):
                        o = ex_pool.tile([P, FD], bf16, name="ex")
                        nc.scalar.activation(o[:], pss[kt_i][:], AF.Exp,
                                             bias=mbias[:, kt_i:kt_i + 1],
                                             scale=SCALE)
                        ex_sb.append(o)
                else:
                    for kt_i in range(KT):
                        ps = psum_pool.tile([P, FD], f32, name="ps")
                        for e in range(DT):
                            nc.tensor.matmul(
                                ps[:],
                                kt_all[:, e * K + kt_i * P:
                                       e * K + (kt_i + 1) * P],
                                qp_cur[e][:],
                                start=(e == 0), stop=(e == DT - 1),
                            )
                        o = ex_pool.tile([P, FD], bf16, name="ex")
                        nc.scalar.activation(o[:], ps[:], AF.Exp,
                                             bias=mbias[:, kt_i:kt_i + 1],
                                             scale=SCALE)
                        ex_sb.append(o)

                nxt = hi + 1
                if nxt < NH and nxt % QH == 0:
                    emit_batch_dmas(nxt // QH)

                # ---- softmax sum over the 8 k-tiles: SERIAL running sum on
                # the DVE (each add fires as soon as its exp tile lands, so
                # the final sum trails the LAST exp by one add instead of a
                # tree's two pending levels), then a single ones-matmul to
                # reduce+broadcast across partitions.
                run = rs_pool.tile([P, FD], f32, name="sm0")
                nc.vector.tensor_add(run[:], ex_sb[0][:], ex_sb[1][:])
                for i in range(2, KT - 1):
                    nxt_run = rs_pool.tile([P, FD], f32, name=f"sm{i}")
                    nc.vector.tensor_add(nxt_run[:], run[:], ex_sb[i][:])
                    run = nxt_run
                tsum = rs_pool.tile([P, FD], bf16, name="smf")
                nc.vector.tensor_add(tsum[:], run[:], ex_sb[KT - 1][:])

                if nxt < NH:
                    # steady state: MM1 of the next half covers the PE while
                    # the softmax reduce + reciprocal + normalize run on
                    # GpSimd/DVE (the cross-partition sum goes to GpSimd's
                    # partition_all_reduce instead of a PE ones-matmul,
                    # shaving 512 PE cycles per half; its latency is hidden
                    # behind MM1-next + MM3)
                    qp_next = []
                    for et in range(3):
                        qp_next.append(emit_mm1_group(nxt, et))

                    rsum = rs_pool.tile([P, FD], f32, name="rsum")
                    nc.gpsimd.partition_all_reduce(rsum[:], tsum[:], P,
                                                   ReduceOp.add)
                    rs = rs_pool.tile([P, FD], f32, name="rs")
                    nc.vector.reciprocal_approx_fast(rs[:], rsum[:])

                    for et in range(3, DT):
                        qp_next.append(emit_mm1_group(nxt, et))

                    # normalize into ONE contiguous staging tile (hidden
                    # behind MM1-next on the PE), then a single big SWDGE
                    # descriptor on the otherwise-idle GpSimd queue.  Small
                    # per-tile writes (1KB dst lines) only sustain ~74GB/s
                    # and 8 of them per half saturate the in-order sync
                    # queue, starving the next batch's input DMAs.
                    aw_st = aw_pool.tile([P, KT * FD], bf16, name="awst",
                                         bufs=2)
                    for kt_i in range(KT):
                        nc.vector.tensor_mul(
                            aw_st[:, kt_i * FD:(kt_i + 1) * FD],
                            ex_sb[kt_i][:], rs[:])
                    nc.gpsimd.dma_start(
                        aw[b].rearrange("(kt p) q -> p kt q", p=P)[:, :, qs],
                        aw_st[:].rearrange("p (kt q) -> p kt q", q=FD))

                    # MM3: att[q-half, e] on normalized weights, staged per
                    # q-tile into [P, D] so the att descriptor writes full
                    # 2KB dram rows (4 descriptors per half instead of 8)
                    for qt_i in range(FD // P):
                        ao = att_pool.tile([P, D], bf16, name="ao")
                        for ec in range(D // FD):
                            ps = psum_pool.tile([P, FD], f32, name="ps")
                            for kt_i in range(KT):
                                nc.tensor.matmul(
                                    ps[:],
                                    aw_st[:, kt_i * FD + qt_i * P:
                                          kt_i * FD + (qt_i + 1) * P],
                                    v_all[:, kt_i * D + ec * FD:
                                          kt_i * D + (ec + 1) * FD],
                                    start=(kt_i == 0), stop=(kt_i == KT - 1),
                                )
                            nc.vector.tensor_copy(
                                ao[:, ec * FD:(ec + 1) * FD], ps[:])
                        q0 = qh * FD + qt_i * P
                        nc.sync.dma_start(att[b, q0:q0 + P, :], ao[:])

                    qp_cur = qp_next
                else:
                    # last half: there is no next MM1 to hide the softmax
                    # critical path behind, so run MM3 on the UNNORMALIZED
                    # weights immediately (only depends on exp), and fold
                    # the 1/sum scale into the SBUF epilogue as a
                    # per-q-partition multiply.  The drain of this half IS
                    # the kernel tail, so everything is staged into few big
                    # output descriptors (descriptor issue costs ~0.6us
                    # apiece on the queue engine) and the aw normalize is
                    # split across DVE+GpSimd with its DMA on the (idle)
                    # GpSimd SWDGE queue.
                    def emit_mm3u_ps(qt_i, ec):
                        ps = psum_pool.tile([P, FD], f32, name="ps")
                        for kt_i in range(KT):
                            nc.tensor.matmul(
                                ps[:],
                                ex_sb[kt_i][:, qt_i * P:(qt_i + 1) * P],
                                v_all[:, kt_i * D + ec * FD:
                                      kt_i * D + (ec + 1) * FD],
                                start=(kt_i == 0), stop=(kt_i == KT - 1),
                            )
                        return ps

                    groups = [(qt_i, ec) for qt_i in range(FD // P)
                              for ec in range(D // FD)]
                    sts = []
                    # one MM3 group first: by its end the DVE add-tree has
                    # produced tsum, so the sum-matmul below runs gap-free.
                    ps0 = emit_mm3u_ps(*groups[0])
                    st0 = st_pool.tile([P, FD], f32, name="st")
                    nc.vector.tensor_copy(st0[:], ps0[:])
                    sts.append((st0,) + groups[0])

                    # softmax scale factors, emitted early so the PE computes
                    # them between MM3 groups and the DVE-side scaling +
                    # output DMA overlaps the remaining MM3 groups
                    ps = psum_pool.tile([P, FD], f32, name="ps")
                    nc.tensor.matmul(ps[:], ones_sb[:], tsum[:],
                                     start=True, stop=True)
                    rs = rs_pool.tile([P, FD], f32, name="rs")
                    nc.vector.reciprocal_approx_fast(rs[:], ps[:])

                    ps1 = emit_mm3u_ps(*groups[1])
                    st1 = st_pool.tile([P, FD], f32, name="st")
                    nc.vector.tensor_copy(st1[:], ps1[:])
                    sts.append((st1,) + groups[1])

                    # weights normalize into ONE contiguous staging tile,
                    # all on the DVE (GpSimd tensor ops are ~4x slower and
                    # would gate the aw descriptor), emitted BEFORE the rc
                    # copies/att scale-outs so the big SWDGE aw descriptor
                    # generates as early as possible -- the end-of-kernel
                    # barrier waits for this transfer.  NOT in place: later
                    # MM3 groups still read the unnormalized ex tiles.
                    aw_st = aw_pool.tile([P, KT * FD], bf16, name="awst",
                                         bufs=2)
                    for kt_i in range(KT):
                        nc.vector.tensor_mul(
                            aw_st[:, kt_i * FD:(kt_i + 1) * FD],
                            ex_sb[kt_i][:], rs[:])
                    nc.gpsimd.dma_start(
                        aw[b].rearrange("(kt p) q -> p kt q", p=P)[:, :, qs],
                        aw_st[:].rearrange("p (kt q) -> p kt q", q=FD))

                    # per-q-partition 1/sum COLUMNS for the att scale: a
                    # tiny N=1 matmul per q-tile, (tsum_slice)^T @ ones_col
                    # = sum(q) in column layout (~60 cycles each, vs ~660
                    # for a PE transpose of the row-form reciprocal), then
                    # a [P,1] reciprocal on the DVE
                    rc_sb = []
                    for qt_i in range(FD // P):
                        pst = psum_pool.tile([P, FD], f32, name="ps")
                        nc.tensor.matmul(pst[:, 0:1],
                                         tsum[:, qt_i * P:(qt_i + 1) * P],
                                         ones_sb[:, 0:1],
                                         start=True, stop=True)
                        rc = rs_pool.tile([P, 1], f32, name="rc", bufs=4)
                        nc.vector.reciprocal_approx_fast(rc[:], pst[:, 0:1])
                        rc_sb.append(rc)

                    # att staged per q-tile ([128, D] covering both ec
                    # halves) -> 4 descriptors of 256KB instead of 8x128KB
                    att_st = [att_pool.tile([P, D], bf16, name="ao")
                              for i in range(FD // P)]

                    def emit_scale_out(src, qt_i, ec):
                        nc.vector.tensor_scalar_mul(
                            att_st[qt_i][:, ec * FD:(ec + 1) * FD], src[:],
                            rc_sb[qt_i][:, 0:1])
                        if ec == D // FD - 1:
                            q0 = qh * FD + qt_i * P
                            nc.sync.dma_start(att[b, q0:q0 + P, :],
                                              att_st[qt_i][:])

                    for st, qt_i, ec in sts:
                        emit_scale_out(st, qt_i, ec)
                    for qt_i, ec in groups[2:]:
                        # rc is ready by now: scale straight out of PSUM in
                        # a single DVE pass (no staging copy)
                        ps = emit_mm3u_ps(qt_i, ec)
                        emit_scale_out(ps, qt_i, ec)
    nc.compile()
    return nc


def _get_nc():
    if "nc" not in _CACHE:
        _CACHE["nc"] = _build_nc()
    return _CACHE["nc"]


def _make_in_maps(query, key, value, mask, W_w, W_b):
    import ml_dtypes
    bf16 = ml_dtypes.bfloat16

    query = np.asarray(query, dtype=np.float32)
    key = np.asarray(key, dtype=np.float32)
    value = np.asarray(value, dtype=np.float32)
    W_w = np.asarray(W_w, dtype=np.float32)
    W_b = np.ascontiguousarray(W_b, dtype=np.float32)
    mask_f = np.ascontiguousarray(mask, dtype=np.float32)

    # host-side layout prep only (transposes / casts, no arithmetic):
    # [B, P, tiles*cols] pre-tiled layouts, see _build_nc
    qT = np.ascontiguousarray(
        query.transpose(0, 2, 1).reshape(B, D // P, P, Q)
        .transpose(0, 2, 1, 3).reshape(B, P, (D // P) * Q)).astype(bf16)
    kT = np.ascontiguousarray(
        key.transpose(0, 2, 1).reshape(B, D // P, P, K)
        .transpose(0, 2, 1, 3).reshape(B, P, (D // P) * K)).astype(bf16)
    v_b = np.ascontiguousarray(
        value.reshape(B, K // P, P, D)
        .transpose(0, 2, 1, 3).reshape(B, P, (K // P) * D)).astype(bf16)
    wT = np.ascontiguousarray(W_w.T).astype(bf16)                  # [Din,Dout]

    in_maps = []
    for c in range(N_CORES):
        sl = slice(c * BPC, (c + 1) * BPC)
        in_maps.append({
            "qT": qT[sl], "kT": kT[sl], "v": v_b[sl],
            "wT": wT, "bias": W_b, "mask": mask_f[sl],
        })
    return in_maps


def kernel(query, key, value, mask, W_w, W_b):
    from concourse.bass_utils import run_bass_kernel_spmd

    nc = _get_nc()
    in_maps = _make_in_maps(query, key, value, mask, W_w, W_b)

    def _axon_reset():
        try:
            import ctypes
            lib = ctypes.CDLL("/opt/axon/libaxon_pjrt.so")
            if hasattr(lib, "axon_reset"):
                lib.axon_reset.restype = ctypes.c_int64
                lib.axon_reset()
        except Exception:
            pass

    att = weights = None
    for _attempt in range(3):
        try:
            res = run_bass_kernel_spmd(nc, in_maps,
                                       core_ids=list(range(N_CORES)))
        except Exception:
            if _attempt == 2:
                raise
            _axon_reset()
            continue
        att = np.concatenate(
            [res.results[c]["att"].astype(np.float32) for c in range(N_CORES)],
            axis=0)
        awT = np.concatenate(
            [res.results[c]["aw"].astype(np.float32) for c in range(N_CORES)],
            axis=0)
        weights = np.ascontiguousarray(awT.transpose(0, 2, 1))  # [B, Q, K]
        # sanity check (guards against rare cold-start misexecution):
        # sampled softmax rows must sum to ~1 and outputs must be finite
        row_sums = weights[:, ::97, :].sum(axis=-1)
        if (np.all(np.abs(row_sums - 1.0) < 5e-2)
                and np.isfinite(att).all()):
            break
    return att, weights



# revision 28
# speedup vs baseline: 1.0049x; 1.0007x over previous
"""Cross-attention kernel for Trainium2 (8 NeuronCores, Bass/Tile).

Reference computation (per batch b):
    qproj = query @ W_w.T + W_b          [Q, D]
    scores = qproj @ key.T * (1/sqrt(D)) [Q, K]
    scores = where(mask==0, -inf, scores)
    w = softmax(scores, axis=-1)         [Q, K]
    att = w @ value                      [Q, D]
    returns (att, w)

Sharding: data-parallel over batch (32 batches -> 4 per core x 8 cores).

Device-side layout strategy (everything chains through the PE with no
on-device transposes; matmuls run in bf16, which keeps the PE at the
full 1 column/cycle rate AND enables the compiler's fast-weight-load
path, unlike fp32r whose 4-byte weight loads bound the matmul issue
period; fp32 accumulation in PSUM throughout):
    MM1: qprojT[e,q] = (W^T as lhsT) . (query^T as rhs), bias added in the
         PSUM->SBUF epilogue (per-partition bias broadcast).
    MM2: scoresT[k,q] = (key^T as lhsT) . qprojT, exp fused into the
         epilogue (ScalarE activation with scale=1/sqrt(D), additive mask
         bias per k-partition).
    softmax sum over k: serial running sum across the 8 k-tiles on the DVE
         (each add fires as its exp tile lands), then cross-partition
         reduce+broadcast via GpSimd partition_all_reduce (steady halves,
         latency hidden) or a PE ones-matmul (last half, latency critical);
         reciprocal on VectorE; normalize into a contiguous staging tile
         written out as ONE big SWDGE descriptor per half on the GpSimd
         queue (small per-tile HWDGE writes with 1KB dst lines only
         sustain ~74GB/s and clog the in-order sync queue).
    MM3: att[q,e] = (wT as lhsT) . (value natural layout as rhs), staged
         per q-tile into [128, D] so att descriptors write full 2KB rows.
    A ~3.4us burst of dummy warm-up matmuls at program start flips the PE
    HAM clock gate (1.2 -> 2.4 GHz) before the real stream begins.

The host side only reshapes/transposes/casts (no arithmetic): query/key
are fed pre-transposed per batch in bf16, W is fed transposed, attention
weights come back transposed [K,Q] in bf16 and are untransposed+upcast on
the host.
"""

import numpy as np

B, Q, K, D = 32, 1024, 1024, 1024
N_CORES = 8
BPC = B // N_CORES          # batches per core
SCALE = 1.0 / float(D) ** 0.5
P = 128                     # SBUF partitions
FD = 512                    # matmul moving free dim (psum bank, fp32 out)
QH = Q // FD                # q processed in halves of 512
NEG_BIG = 1.0e30

_CACHE = {}


def _build_nc():
    import concourse.tile as tile
    from concourse import bacc, mybir
    from concourse.bass_isa import ReduceOp

    f32 = mybir.dt.float32
    bf16 = mybir.dt.bfloat16
    AF = mybir.ActivationFunctionType
    ALU = mybir.AluOpType

    nc = bacc.Bacc("TRN2", target_bir_lowering=False, debug=False,
                   num_devices=N_CORES)

    # host pre-tiled layouts: [P, tile*cols] so every DMA is a plain 2D
    # copy with long contiguous per-partition rows (qT partition p, column
    # d*Q+q holds query[b, q, d*128+p], etc.)
    qT = nc.dram_tensor("qT", [BPC, P, (D // P) * Q], bf16,
                        kind="ExternalInput").ap()
    kT = nc.dram_tensor("kT", [BPC, P, (D // P) * K], bf16,
                        kind="ExternalInput").ap()
    v = nc.dram_tensor("v", [BPC, P, (K // P) * D], bf16,
                       kind="ExternalInput").ap()
    wT = nc.dram_tensor("wT", [D, D], bf16, kind="ExternalInput").ap()
    bias = nc.dram_tensor("bias", [D], f32, kind="ExternalInput").ap()
    mask = nc.dram_tensor("mask", [BPC, K], f32, kind="ExternalInput").ap()
    att = nc.dram_tensor("att", [BPC, Q, D], bf16, kind="ExternalOutput").ap()
    aw = nc.dram_tensor("aw", [BPC, K, Q], bf16, kind="ExternalOutput").ap()

    DT = D // P   # d/e/k tiles of 128
    KT = K // P

    with tile.TileContext(nc) as tc:
        with (
            tc.tile_pool(name="consts", bufs=1) as consts,
            tc.tile_pool(name="wt", bufs=DT) as wt_pool,
            tc.tile_pool(name="qt", bufs=DT) as qt_pool,
            tc.tile_pool(name="kt", bufs=2) as kt_pool,
            tc.tile_pool(name="vv", bufs=2) as v_pool,
            tc.tile_pool(name="qp", bufs=DT) as qp_pool,
            tc.tile_pool(name="ex", bufs=KT) as ex_pool,
            tc.tile_pool(name="mb", bufs=2) as mb_pool,
            tc.tile_pool(name="rs", bufs=2) as rs_pool,
            tc.tile_pool(name="st", bufs=6) as st_pool,
            tc.tile_pool(name="aw", bufs=1) as aw_pool,
            tc.tile_pool(name="ao", bufs=8) as att_pool,
            tc.tile_pool(name="psum", bufs=8, space="PSUM") as psum_pool,
        ):
            # constants: bias (per-partition layout), ones for the
            # partition-dim softmax sum.  Batch 0's W^T/q/k tiles are DMA'd
            # interleaved (emit_batch_dmas below) so the d-outer first MM1
            # starts after <1MB of DMA and MM2 isn't starved on kT.
            # ones tile via a direct bf16 GpSimd memset, emitted FIRST so
            # the PE warm-up below is not gated on the Vector engine's
            # late preamble (memset bit-packs 1.0 per-dtype, so a direct
            # bf16 memset is exact)
            ones_sb = consts.tile([P, P], bf16, name="ones")
            nc.gpsimd.memset(ones_sb[:], 1.0)

            # HAM warm-up: the PE clock-gate sits at 4/8 (1.2 GHz) until it
            # has seen ~3.4us of sustained matmul activity.  The real MM1
            # can't start until ~3us of wT/qT DMA has landed, so burn that
            # wait on dummy matmuls (memset tile, never read) to (a) flip
            # the HAM to 8/8 before real work and (b) not leave the PE idle.
            warm_ps = psum_pool.tile([P, FD], f32, name="ps")
            for _ in range(32):
                nc.tensor.matmul(warm_ps[:, 0:P], ones_sb[:], ones_sb[:],
                                 start=True, stop=True)

            wt_sb = []
            bias_sb = consts.tile([P, DT], f32, name="bias")
            nc.gpsimd.dma_start(bias_sb[:], bias.rearrange("(a b) -> b a", b=P))

            NH = BPC * QH          # total half-iterations on this core

            state = {}             # per-batch tiles: qt/kt/v/mbias

            def emit_batch_dmas(b):
                # tiny mask transfer goes on the GpSimd DMA queue so it is
                # not stuck behind this batch's bulk input DMA
                mraw = mb_pool.tile([P, KT], f32, name="mraw")
                nc.gpsimd.dma_start(mraw[:], mask[b].rearrange("(a b) -> b a", b=P))
                mbias = mb_pool.tile([P, KT], f32, name="mbias")
                nc.vector.tensor_scalar(mbias[:], mraw[:], NEG_BIG, -NEG_BIG,
                                        ALU.mult, ALU.add)
                # kT and v move as ONE descriptor each (descriptor issue on
                # the queue engine costs ~0.6us apiece, so 8 separate tile
                # DMAs would delay MM2's inputs by ~5us at startup); wT/qT
                # stay per-tile so the d-outer MM1 starts on the first pair.
                qt_sb = []
                kt_all = kt_pool.tile([P, DT * K], bf16, name="kt")
                if b == 0:
                    # startup-critical order: (wt[d], qt[d]-first-half) pairs
                    # pace the d-outer MM1 for half 0 (descriptor issue on the
                    # queue engine is ~0.6us apiece, so halving the qt payload
                    # per pair pulls the first matmul ~2us earlier); kt tiles
                    # follow for the e-outer MM2, then the qt second halves
                    # (first needed by MM1 of half 1, ~50us in), then v.
                    for d in range(DT):
                        t = wt_pool.tile([P, D], bf16, name="wt")
                        nc.sync.dma_start(t[:], wT[d * P:(d + 1) * P, :])
                        wt_sb.append(t)
                        t = qt_pool.tile([P, Q], bf16, name="qt")
                        nc.sync.dma_start(t[:, 0:FD], qT[b, :, d * Q:d * Q + FD])
                        qt_sb.append(t)
                    for e in range(DT):
                        nc.sync.dma_start(kt_all[:, e * K:(e + 1) * K],
                                          kT[b, :, e * K:(e + 1) * K])
                    for d in range(DT):
                        nc.sync.dma_start(qt_sb[d][:, FD:Q],
                                          qT[b, :, d * Q + FD:(d + 1) * Q])
                else:
                    nc.sync.dma_start(kt_all[:], kT[b])
                    for d in range(DT):
                        t = qt_pool.tile([P, Q], bf16, name="qt")
                        nc.sync.dma_start(t[:], qT[b, :, d * Q:(d + 1) * Q])
                        qt_sb.append(t)
                v_all = v_pool.tile([P, KT * D], bf16, name="vv")
                nc.sync.dma_start(v_all[:], v[b])
                state[b] = (qt_sb, kt_all, v_all, mbias)

            def emit_mm1_group(hi, et):
                """One MM1 accumulation group: qprojT[e-tile et, half hi]."""
                b, qh = divmod(hi, QH)
                qt_sb = state[b][0]
                qs = slice(qh * FD, (qh + 1) * FD)
                ps = psum_pool.tile([P, FD], f32, name="ps")
                for d in range(DT):
                    nc.tensor.matmul(
                        ps[:],
                        wt_sb[d][:, et * P:(et + 1) * P],
                        qt_sb[d][:, qs],
                        start=(d == 0), stop=(d == DT - 1),
                    )
                o = qp_pool.tile([P, FD], bf16, name="qp")
                nc.scalar.activation(o[:], ps[:], AF.Identity,
                                     bias=bias_sb[:, et:et + 1], scale=1.0)
                return o

            def emit_mm1_douter(hi):
                """MM1 with the d-contraction as the outer loop: the first
                matmuls only need the first wT/qT d-tiles, so the PE starts
                as soon as ~0.75MB of DMA has landed (used for hi==0)."""
                b, qh = divmod(hi, QH)
                qt_sb = state[b][0]
                qs = slice(qh * FD, (qh + 1) * FD)
                pss = [psum_pool.tile([P, FD], f32, name="ps")
                       for _ in range(DT)]
                for d in range(DT):
                    for et in range(DT):
                        nc.tensor.matmul(
                            pss[et][:],
                            wt_sb[d][:, et * P:(et + 1) * P],
                            qt_sb[d][:, qs],
                            start=(d == 0), stop=(d == DT - 1),
                        )
                qp_sb = []
                for et in range(DT):
                    o = qp_pool.tile([P, FD], bf16, name="qp")
                    nc.scalar.activation(o[:], pss[et][:], AF.Identity,
                                         bias=bias_sb[:, et:et + 1], scale=1.0)
                    qp_sb.append(o)
                return qp_sb

            # ---------------- software-pipelined half-iterations ----------
            # PE stream per half hi:
            #   MM2(hi), [MM1(hi+1) et0], sum(hi), [MM1(hi+1) et1..7], MM3(hi)
            # so the reciprocal+normalize of half hi runs on the DVE while
            # the PE chews through MM1 of half hi+1 -> no PE gaps.
            emit_batch_dmas(0)
            qp_cur = emit_mm1_douter(0)

            for hi in range(NH):
                b, qh = divmod(hi, QH)
                qs = slice(qh * FD, (qh + 1) * FD)
                _, kt_all, v_all, mbias = state[b]

                # ---- MM2: scoresT[k, q-half] -> exp ----
                ex_sb = []
                if hi == 0:
                    # e-outer accumulation across all 8 PSUM banks: each
                    # e-pass needs only kt tile e, so MM2 starts while the
                    # tail of the kT DMA is still in flight.
                    pss = [psum_pool.tile([P, FD], f32, name="ps")
                           for _ in range(KT)]
                    for e in range(DT):
                        for kt_i in range(KT):
                            nc.tensor.matmul(
                                pss[kt_i][:],
                                kt_all[:, e * K + kt_i * P:
                                       e * K + (kt_i + 1) * P],
                                qp_cur[e][:],
                                start=(e == 0), stop=(e == DT - 1),
                            )
                    for kt_i in range(KT):
                        o = ex_pool.tile([P, FD], bf16, name="ex")
                        nc.scalar.activation(o[:], pss[kt_i][:], AF.Exp,
                                             bias=mbias[:, kt_i:kt_i + 1],
                                             scale=SCALE)
                        ex_sb.append(o)
                else:
                    for kt_i in range(KT):
                        ps = psum_pool.tile([P, FD], f32, name="ps")
                        for e in range(DT):
                            nc.tensor.matmul(
                                ps[:],
                                kt_all[:, e * K + kt_i * P:
                                       e * K + (kt_i + 1) * P],
                                qp_cur[e][:],
                                start=(e == 0), stop=(e == DT - 1),
                            )
                        o = ex_pool.tile([P, FD], bf16, name="ex")
                        nc.scalar.activation(o[:], ps[:], AF.Exp,
                                             bias=mbias[:, kt_i:kt_i + 1],
                                             scale=SCALE)
                        ex_sb.append(o)

                nxt = hi + 1
                if nxt < NH and nxt % QH == 0:
                    emit_batch_dmas(nxt // QH)

                # ---- softmax sum over the 8 k-tiles: SERIAL running sum on
                # the DVE (each add fires as soon as its exp tile lands, so
                # the final sum trails the LAST exp by one add instead of a
                # tree's two pending levels), then a single ones-matmul to
                # reduce+broadcast across partitions.
                run = rs_pool.tile([P, FD], f32, name="sm0")
                nc.vector.tensor_add(run[:], ex_sb[0][:], ex_sb[1][:])
                for i in range(2, KT - 1):
                    nxt_run = rs_pool.tile([P, FD], f32, name=f"sm{i}")
                    nc.vector.tensor_add(nxt_run[:], run[:], ex_sb[i][:])
                    run = nxt_run
                tsum = rs_pool.tile([P, FD], bf16, name="smf")
                nc.vector.tensor_add(tsum[:], run[:], ex_sb[KT - 1][:])

                if nxt < NH:
                    # steady state: MM1 of the next half covers the PE while
                    # the softmax reduce + reciprocal + normalize run on
                    # GpSimd/DVE (the cross-partition sum goes to GpSimd's
                    # partition_all_reduce instead of a PE ones-matmul,
                    # shaving 512 PE cycles per half; its latency is hidden
                    # behind MM1-next + MM3)
                    qp_next = []
                    for et in range(3):
                        qp_next.append(emit_mm1_group(nxt, et))

                    rsum = rs_pool.tile([P, FD], f32, name="rsum")
                    nc.gpsimd.partition_all_reduce(rsum[:], tsum[:], P,
                                                   ReduceOp.add)
                    rs = rs_pool.tile([P, FD], f32, name="rs")
                    nc.vector.reciprocal_approx_fast(rs[:], rsum[:])

                    for et in range(3, DT):
                        qp_next.append(emit_mm1_group(nxt, et))

                    # normalize into ONE contiguous staging tile (hidden
                    # behind MM1-next on the PE), then a single big SWDGE
                    # descriptor on the otherwise-idle GpSimd queue.  Small
                    # per-tile writes (1KB dst lines) only sustain ~74GB/s
                    # and 8 of them per half saturate the in-order sync
                    # queue, starving the next batch's input DMAs.
                    aw_st = aw_pool.tile([P, KT * FD], bf16, name="awst",
                                         bufs=2)
                    for kt_i in range(KT):
                        nc.vector.tensor_mul(
                            aw_st[:, kt_i * FD:(kt_i + 1) * FD],
                            ex_sb[kt_i][:], rs[:])
                    nc.gpsimd.dma_start(
                        aw[b].rearrange("(kt p) q -> p kt q", p=P)[:, :, qs],
                        aw_st[:].rearrange("p (kt q) -> p kt q", q=FD))

                    # MM3: att[q-half, e] on normalized weights, staged per
                    # q-tile into [P, D] so the att descriptor writes full
                    # 2KB dram rows (4 descriptors per half instead of 8)
                    for qt_i in range(FD // P):
                        ao = att_pool.tile([P, D], bf16, name="ao")
                        for ec in range(D // FD):
                            ps = psum_pool.tile([P, FD], f32, name="ps")
                            for kt_i in range(KT):
                                nc.tensor.matmul(
                                    ps[:],
                                    aw_st[:, kt_i * FD + qt_i * P:
                                          kt_i * FD + (qt_i + 1) * P],
                                    v_all[:, kt_i * D + ec * FD:
                                          kt_i * D + (ec + 1) * FD],
                                    start=(kt_i == 0), stop=(kt_i == KT - 1),
                                )
                            nc.vector.tensor_copy(
                                ao[:, ec * FD:(ec + 1) * FD], ps[:])
                        q0 = qh * FD + qt_i * P
                        nc.sync.dma_start(att[b, q0:q0 + P, :], ao[:])

                    qp_cur = qp_next
                else:
                    # last half: there is no next MM1 to hide the softmax
                    # critical path behind, so run MM3 on the UNNORMALIZED
                    # weights immediately (only depends on exp), and fold
                    # the 1/sum scale into the SBUF epilogue as a
                    # per-q-partition multiply.  The drain of this half IS
                    # the kernel tail, so everything is staged into few big
                    # output descriptors (descriptor issue costs ~0.6us
                    # apiece on the queue engine) and the aw normalize is
                    # split across DVE+GpSimd with its DMA on the (idle)
                    # GpSimd SWDGE queue.
                    def emit_mm3u_ps(qt_i, ec):
                        ps = psum_pool.tile([P, FD], f32, name="ps")
                        for kt_i in range(KT):
                            nc.tensor.matmul(
                                ps[:],
                                ex_sb[kt_i][:, qt_i * P:(qt_i + 1) * P],
                                v_all[:, kt_i * D + ec * FD:
                                      kt_i * D + (ec + 1) * FD],
                                start=(kt_i == 0), stop=(kt_i == KT - 1),
                            )
                        return ps

                    groups = [(qt_i, ec) for qt_i in range(FD // P)
                              for ec in range(D // FD)]
                    sts = []
                    # one MM3 group first: by its end the DVE add-tree has
                    # produced tsum, so the sum-matmul below runs gap-free.
                    ps0 = emit_mm3u_ps(*groups[0])
                    st0 = st_pool.tile([P, FD], f32, name="st")
                    nc.vector.tensor_copy(st0[:], ps0[:])
                    sts.append((st0,) + groups[0])

                    # softmax scale factors, emitted early so the PE computes
                    # them between MM3 groups and the DVE-side scaling +
                    # output DMA overlaps the remaining MM3 groups
                    ps = psum_pool.tile([P, FD], f32, name="ps")
                    nc.tensor.matmul(ps[:], ones_sb[:], tsum[:],
                                     start=True, stop=True)
                    rs = rs_pool.tile([P, FD], f32, name="rs")
                    nc.vector.reciprocal_approx_fast(rs[:], ps[:])

                    ps1 = emit_mm3u_ps(*groups[1])
                    st1 = st_pool.tile([P, FD], f32, name="st")
                    nc.vector.tensor_copy(st1[:], ps1[:])
                    sts.append((st1,) + groups[1])

                    # weights normalize into ONE contiguous staging tile,
                    # all on the DVE (GpSimd tensor ops are ~4x slower and
                    # would gate the aw descriptor), emitted BEFORE the rc
                    # copies/att scale-outs so the big SWDGE aw descriptor
                    # generates as early as possible -- the end-of-kernel
                    # barrier waits for this transfer.  NOT in place: later
                    # MM3 groups still read the unnormalized ex tiles.
                    aw_st = aw_pool.tile([P, KT * FD], bf16, name="awst",
                                         bufs=2)
                    for kt_i in range(KT):
                        nc.vector.tensor_mul(
                            aw_st[:, kt_i * FD:(kt_i + 1) * FD],
                            ex_sb[kt_i][:], rs[:])
                    nc.gpsimd.dma_start(
                        aw[b].rearrange("(kt p) q -> p kt q", p=P)[:, :, qs],
                        aw_st[:].rearrange("p (kt q) -> p kt q", q=FD))

                    # per-q-partition 1/sum COLUMNS for the att scale: a
                    # tiny N=1 matmul per q-tile, (tsum_slice)^T @ ones_col
                    # = sum(q) in column layout (~60 cycles each, vs ~660
                    # for a PE transpose of the row-form reciprocal), then
                    # a [P,1] reciprocal on the DVE
                    rc_sb = []
                    for qt_i in range(FD // P):
                        pst = psum_pool.tile([P, FD], f32, name="ps")
                        nc.tensor.matmul(pst[:, 0:1],
                                         tsum[:, qt_i * P:(qt_i + 1) * P],
                                         ones_sb[:, 0:1],
                                         start=True, stop=True)
                        rc = rs_pool.tile([P, 1], f32, name="rc", bufs=4)
                        nc.vector.reciprocal_approx_fast(rc[:], pst[:, 0:1])
                        rc_sb.append(rc)

                    # att staged per q-tile ([128, D] covering both ec
                    # halves) -> 4 descriptors of 256KB instead of 8x128KB
                    att_st = [att_pool.tile([P, D], bf16, name="ao")
                              for i in range(FD // P)]

                    def emit_scale_out(src, qt_i, ec):
                        nc.vector.tensor_scalar_mul(
                            att_st[qt_i][:, ec * FD:(ec + 1) * FD], src[:],
                            rc_sb[qt_i][:, 0:1])
                        if ec == D // FD - 1:
                            q0 = qh * FD + qt_i * P
                            nc.sync.dma_start(att[b, q0:q0 + P, :],
                                              att_st[qt_i][:])

                    for st, qt_i, ec in sts:
                        emit_scale_out(st, qt_i, ec)
                    for qt_i, ec in groups[2:]:
                        # rc is ready by now: scale straight out of PSUM in
                        # a single DVE pass (no staging copy)
                        ps = emit_mm3u_ps(qt_i, ec)
                        emit_scale_out(ps, qt_i, ec)
    nc.compile()
    return nc


def _get_nc():
    if "nc" not in _CACHE:
        _CACHE["nc"] = _build_nc()
    return _CACHE["nc"]


def _make_in_maps(query, key, value, mask, W_w, W_b):
    import ml_dtypes
    bf16 = ml_dtypes.bfloat16

    query = np.asarray(query, dtype=np.float32)
    key = np.asarray(key, dtype=np.float32)
    value = np.asarray(value, dtype=np.float32)
    W_w = np.asarray(W_w, dtype=np.float32)
    W_b = np.ascontiguousarray(W_b, dtype=np.float32)
    mask_f = np.ascontiguousarray(mask, dtype=np.float32)

    # host-side layout prep only (transposes / casts, no arithmetic):
    # [B, P, tiles*cols] pre-tiled layouts, see _build_nc
    qT = np.ascontiguousarray(
        query.transpose(0, 2, 1).reshape(B, D // P, P, Q)
        .transpose(0, 2, 1, 3).reshape(B, P, (D // P) * Q)).astype(bf16)
    kT = np.ascontiguousarray(
        key.transpose(0, 2, 1).reshape(B, D // P, P, K)
        .transpose(0, 2, 1, 3).reshape(B, P, (D // P) * K)).astype(bf16)
    v_b = np.ascontiguousarray(
        value.reshape(B, K // P, P, D)
        .transpose(0, 2, 1, 3).reshape(B, P, (K // P) * D)).astype(bf16)
    wT = np.ascontiguousarray(W_w.T).astype(bf16)                  # [Din,Dout]

    in_maps = []
    for c in range(N_CORES):
        sl = slice(c * BPC, (c + 1) * BPC)
        in_maps.append({
            "qT": qT[sl], "kT": kT[sl], "v": v_b[sl],
            "wT": wT, "bias": W_b, "mask": mask_f[sl],
        })
    return in_maps


def kernel(query, key, value, mask, W_w, W_b):
    from concourse.bass_utils import run_bass_kernel_spmd

    nc = _get_nc()
    in_maps = _make_in_maps(query, key, value, mask, W_w, W_b)

    def _axon_reset():
        try:
            import ctypes
            lib = ctypes.CDLL("/opt/axon/libaxon_pjrt.so")
            if hasattr(lib, "axon_reset"):
                lib.axon_reset.restype = ctypes.c_int64
                lib.axon_reset()
        except Exception:
            pass

    att = weights = None
    for _attempt in range(3):
        try:
            res = run_bass_kernel_spmd(nc, in_maps,
                                       core_ids=list(range(N_CORES)))
        except Exception:
            if _attempt == 2:
                raise
            _axon_reset()
            continue
        att = np.concatenate(
            [res.results[c]["att"].astype(np.float32) for c in range(N_CORES)],
            axis=0)
        awT = np.concatenate(
            [res.results[c]["aw"].astype(np.float32) for c in range(N_CORES)],
            axis=0)
        weights = np.ascontiguousarray(awT.transpose(0, 2, 1))  # [B, Q, K]
        # sanity check (guards against rare cold-start misexecution):
        # sampled softmax rows must sum to ~1 and outputs must be finite
        row_sums = weights[:, ::97, :].sum(axis=-1)
        if (np.all(np.abs(row_sums - 1.0) < 5e-2)
                and np.isfinite(att).all()):
            break
    return att, weights



# revision 29
# speedup vs baseline: 1.0084x; 1.0034x over previous
"""Cross-attention kernel for Trainium2 (8 NeuronCores, Bass/Tile).

Reference computation (per batch b):
    qproj = query @ W_w.T + W_b          [Q, D]
    scores = qproj @ key.T * (1/sqrt(D)) [Q, K]
    scores = where(mask==0, -inf, scores)
    w = softmax(scores, axis=-1)         [Q, K]
    att = w @ value                      [Q, D]
    returns (att, w)

Sharding: data-parallel over batch (32 batches -> 4 per core x 8 cores).

Device-side layout strategy (everything chains through the PE with no
on-device transposes; matmuls run in bf16, which keeps the PE at the
full 1 column/cycle rate AND enables the compiler's fast-weight-load
path, unlike fp32r whose 4-byte weight loads bound the matmul issue
period; fp32 accumulation in PSUM throughout):
    MM1: qprojT[e,q] = (W^T as lhsT) . (query^T as rhs), bias added in the
         PSUM->SBUF epilogue (per-partition bias broadcast).
    MM2: scoresT[k,q] = (key^T as lhsT) . qprojT, exp fused into the
         epilogue (ScalarE activation with scale=1/sqrt(D), additive mask
         bias per k-partition).
    softmax sum over k: serial running sum across the 8 k-tiles on the DVE
         (each add fires as its exp tile lands), then cross-partition
         reduce+broadcast via GpSimd partition_all_reduce (steady halves,
         latency hidden) or a PE ones-matmul (last half, latency critical);
         reciprocal on VectorE; normalize into a contiguous staging tile
         written out as ONE big SWDGE descriptor per half on the GpSimd
         queue (small per-tile HWDGE writes with 1KB dst lines only
         sustain ~74GB/s and clog the in-order sync queue).
    MM3: att[q,e] = (wT as lhsT) . (value natural layout as rhs), staged
         per q-tile into [128, D] so att descriptors write full 2KB rows.
    A ~3.4us burst of dummy warm-up matmuls at program start flips the PE
    HAM clock gate (1.2 -> 2.4 GHz) before the real stream begins.

The host side only reshapes/transposes/casts (no arithmetic): query/key
are fed pre-transposed per batch in bf16, W is fed transposed, attention
weights come back transposed [K,Q] in bf16 and are untransposed+upcast on
the host.
"""

import numpy as np

B, Q, K, D = 32, 1024, 1024, 1024
N_CORES = 8
BPC = B // N_CORES          # batches per core
SCALE = 1.0 / float(D) ** 0.5
P = 128                     # SBUF partitions
FD = 512                    # matmul moving free dim (psum bank, fp32 out)
QH = Q // FD                # q processed in halves of 512
NEG_BIG = 1.0e30

_CACHE = {}


def _build_nc():
    import concourse.tile as tile
    from concourse import bacc, mybir
    from concourse.bass_isa import ReduceOp

    f32 = mybir.dt.float32
    bf16 = mybir.dt.bfloat16
    AF = mybir.ActivationFunctionType
    ALU = mybir.AluOpType

    nc = bacc.Bacc("TRN2", target_bir_lowering=False, debug=False,
                   num_devices=N_CORES)

    # host pre-tiled layouts: [P, tile*cols] so every DMA is a plain 2D
    # copy with long contiguous per-partition rows (qT partition p, column
    # d*Q+q holds query[b, q, d*128+p], etc.)
    qT = nc.dram_tensor("qT", [BPC, P, (D // P) * Q], bf16,
                        kind="ExternalInput").ap()
    kT = nc.dram_tensor("kT", [BPC, P, (D // P) * K], bf16,
                        kind="ExternalInput").ap()
    v = nc.dram_tensor("v", [BPC, P, (K // P) * D], bf16,
                       kind="ExternalInput").ap()
    wT = nc.dram_tensor("wT", [D, D], bf16, kind="ExternalInput").ap()
    bias = nc.dram_tensor("bias", [D], f32, kind="ExternalInput").ap()
    mask = nc.dram_tensor("mask", [BPC, K], f32, kind="ExternalInput").ap()
    att = nc.dram_tensor("att", [BPC, Q, D], bf16, kind="ExternalOutput").ap()
    aw = nc.dram_tensor("aw", [BPC, K, Q], bf16, kind="ExternalOutput").ap()

    DT = D // P   # d/e/k tiles of 128
    KT = K // P

    with tile.TileContext(nc) as tc:
        with (
            tc.tile_pool(name="consts", bufs=1) as consts,
            tc.tile_pool(name="wt", bufs=DT) as wt_pool,
            tc.tile_pool(name="qt", bufs=DT) as qt_pool,
            tc.tile_pool(name="kt", bufs=2) as kt_pool,
            tc.tile_pool(name="vv", bufs=2) as v_pool,
            tc.tile_pool(name="qp", bufs=DT) as qp_pool,
            tc.tile_pool(name="ex", bufs=KT) as ex_pool,
            tc.tile_pool(name="mb", bufs=2) as mb_pool,
            tc.tile_pool(name="rs", bufs=2) as rs_pool,
            tc.tile_pool(name="st", bufs=6) as st_pool,
            tc.tile_pool(name="aw", bufs=1) as aw_pool,
            tc.tile_pool(name="ao", bufs=8) as att_pool,
            tc.tile_pool(name="psum", bufs=8, space="PSUM") as psum_pool,
        ):
            # constants: bias (per-partition layout), ones for the
            # partition-dim softmax sum.  Batch 0's W^T/q/k tiles are DMA'd
            # interleaved (emit_batch_dmas below) so the d-outer first MM1
            # starts after <1MB of DMA and MM2 isn't starved on kT.
            wt_sb = []
            bias_sb = consts.tile([P, DT], f32, name="bias")
            nc.gpsimd.dma_start(bias_sb[:], bias.rearrange("(a b) -> b a", b=P))
            ones_f = consts.tile([P, P], f32, name="onesf")
            nc.vector.memset(ones_f[:], 1.0)
            ones_sb = consts.tile([P, P], bf16, name="ones")
            nc.vector.tensor_copy(ones_sb[:], ones_f[:])

            # HAM warm-up: the PE clock-gate sits at 4/8 (1.2 GHz) until it
            # has seen ~3.4us of sustained matmul activity.  The real MM1
            # can't start until ~3us of wT/qT DMA has landed, so burn that
            # wait on dummy matmuls (memset tile, never read) to (a) flip
            # the HAM to 8/8 before real work and (b) not leave the PE idle.
            warm_ps = psum_pool.tile([P, FD], f32, name="ps")
            for _ in range(32):
                nc.tensor.matmul(warm_ps[:, 0:P], ones_sb[:], ones_sb[:],
                                 start=True, stop=True)

            NH = BPC * QH          # total half-iterations on this core

            state = {}             # per-batch tiles: qt/kt/v/mbias

            def emit_batch_dmas(b):
                # tiny mask transfer goes on the GpSimd DMA queue so it is
                # not stuck behind this batch's bulk input DMA
                mraw = mb_pool.tile([P, KT], f32, name="mraw")
                nc.gpsimd.dma_start(mraw[:], mask[b].rearrange("(a b) -> b a", b=P))
                mbias = mb_pool.tile([P, KT], f32, name="mbias")
                nc.vector.tensor_scalar(mbias[:], mraw[:], NEG_BIG, -NEG_BIG,
                                        ALU.mult, ALU.add)
                # kT and v move as ONE descriptor each (descriptor issue on
                # the queue engine costs ~0.6us apiece, so 8 separate tile
                # DMAs would delay MM2's inputs by ~5us at startup); wT/qT
                # stay per-tile so the d-outer MM1 starts on the first pair.
                qt_sb = []
                kt_all = kt_pool.tile([P, DT * K], bf16, name="kt")
                if b == 0:
                    # startup-critical order: (wt[d], qt[d]-first-half) pairs
                    # pace the d-outer MM1 for half 0 (descriptor issue on the
                    # queue engine is ~0.6us apiece, so halving the qt payload
                    # per pair pulls the first matmul ~2us earlier); kt tiles
                    # follow for the e-outer MM2, then the qt second halves
                    # (first needed by MM1 of half 1, ~50us in), then v.
                    for d in range(DT):
                        t = wt_pool.tile([P, D], bf16, name="wt")
                        nc.sync.dma_start(t[:], wT[d * P:(d + 1) * P, :])
                        wt_sb.append(t)
                        t = qt_pool.tile([P, Q], bf16, name="qt")
                        nc.sync.dma_start(t[:, 0:FD], qT[b, :, d * Q:d * Q + FD])
                        qt_sb.append(t)
                    for e in range(DT):
                        nc.sync.dma_start(kt_all[:, e * K:(e + 1) * K],
                                          kT[b, :, e * K:(e + 1) * K])
                    for d in range(DT):
                        nc.sync.dma_start(qt_sb[d][:, FD:Q],
                                          qT[b, :, d * Q + FD:(d + 1) * Q])
                else:
                    nc.sync.dma_start(kt_all[:], kT[b])
                    for d in range(DT):
                        t = qt_pool.tile([P, Q], bf16, name="qt")
                        nc.sync.dma_start(t[:], qT[b, :, d * Q:(d + 1) * Q])
                        qt_sb.append(t)
                v_all = v_pool.tile([P, KT * D], bf16, name="vv")
                nc.sync.dma_start(v_all[:], v[b])
                state[b] = (qt_sb, kt_all, v_all, mbias)

            def emit_mm1_group(hi, et):
                """One MM1 accumulation group: qprojT[e-tile et, half hi]."""
                b, qh = divmod(hi, QH)
                qt_sb = state[b][0]
                qs = slice(qh * FD, (qh + 1) * FD)
                ps = psum_pool.tile([P, FD], f32, name="ps")
                for d in range(DT):
                    nc.tensor.matmul(
                        ps[:],
                        wt_sb[d][:, et * P:(et + 1) * P],
                        qt_sb[d][:, qs],
                        start=(d == 0), stop=(d == DT - 1),
                    )
                o = qp_pool.tile([P, FD], bf16, name="qp")
                nc.scalar.activation(o[:], ps[:], AF.Identity,
                                     bias=bias_sb[:, et:et + 1], scale=1.0)
                return o

            def emit_mm1_douter(hi):
                """MM1 with the d-contraction as the outer loop: the first
                matmuls only need the first wT/qT d-tiles, so the PE starts
                as soon as ~0.75MB of DMA has landed (used for hi==0)."""
                b, qh = divmod(hi, QH)
                qt_sb = state[b][0]
                qs = slice(qh * FD, (qh + 1) * FD)
                pss = [psum_pool.tile([P, FD], f32, name="ps")
                       for _ in range(DT)]
                for d in range(DT):
                    for et in range(DT):
                        nc.tensor.matmul(
                            pss[et][:],
                            wt_sb[d][:, et * P:(et + 1) * P],
                            qt_sb[d][:, qs],
                            start=(d == 0), stop=(d == DT - 1),
                        )
                qp_sb = []
                for et in range(DT):
                    o = qp_pool.tile([P, FD], bf16, name="qp")
                    nc.scalar.activation(o[:], pss[et][:], AF.Identity,
                                         bias=bias_sb[:, et:et + 1], scale=1.0)
                    qp_sb.append(o)
                return qp_sb

            # ---------------- software-pipelined half-iterations ----------
            # PE stream per half hi:
            #   MM2(hi), [MM1(hi+1) et0], sum(hi), [MM1(hi+1) et1..7], MM3(hi)
            # so the reciprocal+normalize of half hi runs on the DVE while
            # the PE chews through MM1 of half hi+1 -> no PE gaps.
            emit_batch_dmas(0)
            qp_cur = emit_mm1_douter(0)

            for hi in range(NH):
                b, qh = divmod(hi, QH)
                qs = slice(qh * FD, (qh + 1) * FD)
                _, kt_all, v_all, mbias = state[b]

                # ---- MM2: scoresT[k, q-half] -> exp ----
                ex_sb = []
                if hi == 0:
                    # e-outer accumulation across all 8 PSUM banks: each
                    # e-pass needs only kt tile e, so MM2 starts while the
                    # tail of the kT DMA is still in flight.
                    pss = [psum_pool.tile([P, FD], f32, name="ps")
                           for _ in range(KT)]
                    for e in range(DT):
                        for kt_i in range(KT):
                            nc.tensor.matmul(
                                pss[kt_i][:],
                                kt_all[:, e * K + kt_i * P:
                                       e * K + (kt_i + 1) * P],
                                qp_cur[e][:],
                                start=(e == 0), stop=(e == DT - 1),
                            )
                    for kt_i in range(KT):
                        o = ex_pool.tile([P, FD], bf16, name="ex")
                        nc.scalar.activation(o[:], pss[kt_i][:], AF.Exp,
                                             bias=mbias[:, kt_i:kt_i + 1],
                                             scale=SCALE)
                        ex_sb.append(o)
                else:
                    for kt_i in range(KT):
                        ps = psum_pool.tile([P, FD], f32, name="ps")
                        for e in range(DT):
                            nc.tensor.matmul(
                                ps[:],
                                kt_all[:, e * K + kt_i * P:
                                       e * K + (kt_i + 1) * P],
                                qp_cur[e][:],
                                start=(e == 0), stop=(e == DT - 1),
                            )
                        o = ex_pool.tile([P, FD], bf16, name="ex")
                        nc.scalar.activation(o[:], ps[:], AF.Exp,
                                             bias=mbias[:, kt_i:kt_i + 1],
                                             scale=SCALE)
                        ex_sb.append(o)

                nxt = hi + 1
                if nxt < NH and nxt % QH == 0:
                    emit_batch_dmas(nxt // QH)

                # ---- softmax sum over the 8 k-tiles: SERIAL running sum on
                # the DVE (each add fires as soon as its exp tile lands, so
                # the final sum trails the LAST exp by one add instead of a
                # tree's two pending levels), then a single ones-matmul to
                # reduce+broadcast across partitions.
                run = rs_pool.tile([P, FD], f32, name="sm0")
                nc.vector.tensor_add(run[:], ex_sb[0][:], ex_sb[1][:])
                for i in range(2, KT - 1):
                    nxt_run = rs_pool.tile([P, FD], f32, name=f"sm{i}")
                    nc.vector.tensor_add(nxt_run[:], run[:], ex_sb[i][:])
                    run = nxt_run
                tsum = rs_pool.tile([P, FD], bf16, name="smf")
                nc.vector.tensor_add(tsum[:], run[:], ex_sb[KT - 1][:])

                if nxt < NH:
                    # steady state: MM1 of the next half covers the PE while
                    # the softmax reduce + reciprocal + normalize run on
                    # GpSimd/DVE (the cross-partition sum goes to GpSimd's
                    # partition_all_reduce instead of a PE ones-matmul,
                    # shaving 512 PE cycles per half; its latency is hidden
                    # behind MM1-next + MM3)
                    qp_next = []
                    for et in range(3):
                        qp_next.append(emit_mm1_group(nxt, et))

                    rsum = rs_pool.tile([P, FD], f32, name="rsum")
                    nc.gpsimd.partition_all_reduce(rsum[:], tsum[:], P,
                                                   ReduceOp.add)
                    rs = rs_pool.tile([P, FD], f32, name="rs")
                    nc.vector.reciprocal_approx_fast(rs[:], rsum[:])

                    for et in range(3, DT):
                        qp_next.append(emit_mm1_group(nxt, et))

                    # normalize into ONE contiguous staging tile (hidden
                    # behind MM1-next on the PE), then a single big SWDGE
                    # descriptor on the otherwise-idle GpSimd queue.  Small
                    # per-tile writes (1KB dst lines) only sustain ~74GB/s
                    # and 8 of them per half saturate the in-order sync
                    # queue, starving the next batch's input DMAs.
                    aw_st = aw_pool.tile([P, KT * FD], bf16, name="awst",
                                         bufs=2)
                    for kt_i in range(KT):
                        nc.vector.tensor_mul(
                            aw_st[:, kt_i * FD:(kt_i + 1) * FD],
                            ex_sb[kt_i][:], rs[:])
                    nc.gpsimd.dma_start(
                        aw[b].rearrange("(kt p) q -> p kt q", p=P)[:, :, qs],
                        aw_st[:].rearrange("p (kt q) -> p kt q", q=FD))

                    # MM3: att[q-half, e] on normalized weights, staged per
                    # q-tile into [P, D] so the att descriptor writes full
                    # 2KB dram rows (4 descriptors per half instead of 8)
                    for qt_i in range(FD // P):
                        ao = att_pool.tile([P, D], bf16, name="ao")
                        for ec in range(D // FD):
                            ps = psum_pool.tile([P, FD], f32, name="ps")
                            for kt_i in range(KT):
                                nc.tensor.matmul(
                                    ps[:],
                                    aw_st[:, kt_i * FD + qt_i * P:
                                          kt_i * FD + (qt_i + 1) * P],
                                    v_all[:, kt_i * D + ec * FD:
                                          kt_i * D + (ec + 1) * FD],
                                    start=(kt_i == 0), stop=(kt_i == KT - 1),
                                )
                            nc.vector.tensor_copy(
                                ao[:, ec * FD:(ec + 1) * FD], ps[:])
                        q0 = qh * FD + qt_i * P
                        nc.sync.dma_start(att[b, q0:q0 + P, :], ao[:])

                    qp_cur = qp_next
                else:
                    # last half: there is no next MM1 to hide the softmax
                    # critical path behind, so run MM3 on the UNNORMALIZED
                    # weights immediately (only depends on exp), and fold
                    # the 1/sum scale into the SBUF epilogue as a
                    # per-q-partition multiply.  The drain of this half IS
                    # the kernel tail, so everything is staged into few big
                    # output descriptors (descriptor issue costs ~0.6us
                    # apiece on the queue engine) and the aw normalize is
                    # split across DVE+GpSimd with its DMA on the (idle)
                    # GpSimd SWDGE queue.
                    def emit_mm3u_ps(qt_i, ec):
                        ps = psum_pool.tile([P, FD], f32, name="ps")
                        for kt_i in range(KT):
                            nc.tensor.matmul(
                                ps[:],
                                ex_sb[kt_i][:, qt_i * P:(qt_i + 1) * P],
                                v_all[:, kt_i * D + ec * FD:
                                      kt_i * D + (ec + 1) * FD],
                                start=(kt_i == 0), stop=(kt_i == KT - 1),
                            )
                        return ps

                    groups = [(qt_i, ec) for qt_i in range(FD // P)
                              for ec in range(D // FD)]
                    sts = []
                    # one MM3 group first: by its end the DVE add-tree has
                    # produced tsum, so the sum-matmul below runs gap-free.
                    ps0 = emit_mm3u_ps(*groups[0])
                    st0 = st_pool.tile([P, FD], f32, name="st")
                    nc.vector.tensor_copy(st0[:], ps0[:])
                    sts.append((st0,) + groups[0])

                    # softmax scale factors, emitted early so the PE computes
                    # them between MM3 groups and the DVE-side scaling +
                    # output DMA overlaps the remaining MM3 groups
                    ps = psum_pool.tile([P, FD], f32, name="ps")
                    nc.tensor.matmul(ps[:], ones_sb[:], tsum[:],
                                     start=True, stop=True)
                    rs = rs_pool.tile([P, FD], f32, name="rs")
                    nc.vector.reciprocal_approx_fast(rs[:], ps[:])

                    ps1 = emit_mm3u_ps(*groups[1])
                    st1 = st_pool.tile([P, FD], f32, name="st")
                    nc.vector.tensor_copy(st1[:], ps1[:])
                    sts.append((st1,) + groups[1])

                    # weights normalize into ONE contiguous staging tile,
                    # all on the DVE (GpSimd tensor ops are ~4x slower and
                    # would gate the aw descriptor), emitted BEFORE the rc
                    # copies/att scale-outs so the big SWDGE aw descriptor
                    # generates as early as possible -- the end-of-kernel
                    # barrier waits for this transfer.  NOT in place: later
                    # MM3 groups still read the unnormalized ex tiles.
                    aw_st = aw_pool.tile([P, KT * FD], bf16, name="awst",
                                         bufs=2)
                    for kt_i in range(KT):
                        nc.vector.tensor_mul(
                            aw_st[:, kt_i * FD:(kt_i + 1) * FD],
                            ex_sb[kt_i][:], rs[:])
                    nc.gpsimd.dma_start(
                        aw[b].rearrange("(kt p) q -> p kt q", p=P)[:, :, qs],
                        aw_st[:].rearrange("p (kt q) -> p kt q", q=FD))

                    # per-q-partition 1/sum COLUMNS for the att scale: a
                    # tiny N=1 matmul per q-tile, (tsum_slice)^T @ ones_col
                    # = sum(q) in column layout (~60 cycles each, vs ~660
                    # for a PE transpose of the row-form reciprocal), then
                    # a [P,1] reciprocal on the DVE
                    rc_sb = []
                    for qt_i in range(FD // P):
                        pst = psum_pool.tile([P, FD], f32, name="ps")
                        nc.tensor.matmul(pst[:, 0:1],
                                         tsum[:, qt_i * P:(qt_i + 1) * P],
                                         ones_sb[:, 0:1],
                                         start=True, stop=True)
                        rc = rs_pool.tile([P, 1], f32, name="rc", bufs=4)
                        nc.vector.reciprocal_approx_fast(rc[:], pst[:, 0:1])
                        rc_sb.append(rc)

                    # att staged per q-tile ([128, D] covering both ec
                    # halves) -> 4 descriptors of 256KB instead of 8x128KB
                    att_st = [att_pool.tile([P, D], bf16, name="ao")
                              for i in range(FD // P)]

                    def emit_scale_out(src, qt_i, ec):
                        nc.vector.tensor_scalar_mul(
                            att_st[qt_i][:, ec * FD:(ec + 1) * FD], src[:],
                            rc_sb[qt_i][:, 0:1])
                        if ec == D // FD - 1:
                            q0 = qh * FD + qt_i * P
                            nc.sync.dma_start(att[b, q0:q0 + P, :],
                                              att_st[qt_i][:])

                    for st, qt_i, ec in sts:
                        emit_scale_out(st, qt_i, ec)
                    for qt_i, ec in groups[2:]:
                        # rc is ready by now: scale straight out of PSUM in
                        # a single DVE pass (no staging copy)
                        ps = emit_mm3u_ps(qt_i, ec)
                        emit_scale_out(ps, qt_i, ec)
    nc.compile()
    return nc


def _get_nc():
    if "nc" not in _CACHE:
        _CACHE["nc"] = _build_nc()
    return _CACHE["nc"]


def _make_in_maps(query, key, value, mask, W_w, W_b):
    import ml_dtypes
    bf16 = ml_dtypes.bfloat16

    query = np.asarray(query, dtype=np.float32)
    key = np.asarray(key, dtype=np.float32)
    value = np.asarray(value, dtype=np.float32)
    W_w = np.asarray(W_w, dtype=np.float32)
    W_b = np.ascontiguousarray(W_b, dtype=np.float32)
    mask_f = np.ascontiguousarray(mask, dtype=np.float32)

    # host-side layout prep only (transposes / casts, no arithmetic):
    # [B, P, tiles*cols] pre-tiled layouts, see _build_nc
    qT = np.ascontiguousarray(
        query.transpose(0, 2, 1).reshape(B, D // P, P, Q)
        .transpose(0, 2, 1, 3).reshape(B, P, (D // P) * Q)).astype(bf16)
    kT = np.ascontiguousarray(
        key.transpose(0, 2, 1).reshape(B, D // P, P, K)
        .transpose(0, 2, 1, 3).reshape(B, P, (D // P) * K)).astype(bf16)
    v_b = np.ascontiguousarray(
        value.reshape(B, K // P, P, D)
        .transpose(0, 2, 1, 3).reshape(B, P, (K // P) * D)).astype(bf16)
    wT = np.ascontiguousarray(W_w.T).astype(bf16)                  # [Din,Dout]

    in_maps = []
    for c in range(N_CORES):
        sl = slice(c * BPC, (c + 1) * BPC)
        in_maps.append({
            "qT": qT[sl], "kT": kT[sl], "v": v_b[sl],
            "wT": wT, "bias": W_b, "mask": mask_f[sl],
        })
    return in_maps


def kernel(query, key, value, mask, W_w, W_b):
    from concourse.bass_utils import run_bass_kernel_spmd

    nc = _get_nc()
    in_maps = _make_in_maps(query, key, value, mask, W_w, W_b)

    def _axon_reset():
        try:
            import ctypes
            lib = ctypes.CDLL("/opt/axon/libaxon_pjrt.so")
            if hasattr(lib, "axon_reset"):
                lib.axon_reset.restype = ctypes.c_int64
                lib.axon_reset()
        except Exception:
            pass

    att = weights = None
    for _attempt in range(3):
        try:
            res = run_bass_kernel_spmd(nc, in_maps,
                                       core_ids=list(range(N_CORES)))
        except Exception:
            if _attempt == 2:
                raise
            _axon_reset()
            continue
        att = np.concatenate(
            [res.results[c]["att"].astype(np.float32) for c in range(N_CORES)],
            axis=0)
        awT = np.concatenate(
            [res.results[c]["aw"].astype(np.float32) for c in range(N_CORES)],
            axis=0)
        weights = np.ascontiguousarray(awT.transpose(0, 2, 1))  # [B, Q, K]
        # sanity check (guards against rare cold-start misexecution):
        # sampled softmax rows must sum to ~1 and outputs must be finite
        row_sums = weights[:, ::97, :].sum(axis=-1)
        if (np.all(np.abs(row_sums - 1.0) < 5e-2)
                and np.isfinite(att).all()):
            break
    return att, weights

